# revision 13
# baseline (speedup 1.0000x reference)
"""Trainium2 Bass kernel for a 2-layer LLaMA-style decoder with per-layer
memory K/V prefix (tokenmix2 Decoder), tensor-parallel over 8 NeuronCores.

Sharding: heads (32 -> 4/core), FFN intermediate (8192 -> 1024/core),
vocab (8192 -> 1024/core).  Two AllReduces per layer (attention out,
FFN out), each split into two sequence-chunk collectives for overlap.

Layouts: activations are kept transposed (xT [D, S]) so every matmul
output feeds the next without transposes.  Attention computes
scoresT [t, s] per head; softmax runs without max-subtraction (scores
are ~N(0,1) after the 1/sqrt(Dh) scale) with the normalization applied
on the oT psum evacuation.  Matmul compute in bf16, residual stream and
psum accumulation in fp32.
"""
import sys

sys.path.insert(0, "/opt/trn_rl_repo")

import numpy as np
import ml_dtypes

import concourse.bass as bass
import concourse.mybir as mybir
import concourse.tile as tile
from concourse import bacc
from concourse.bass_utils import run_bass_kernel_spmd

BF = np.float16

# model dims
L, D, H, DH, F, V = 2, 4096, 32, 128, 8192, 8192
B, S, M = 1, 1024, 512
T = M + S                      # 1536 total key positions
EPS = 1e-5
ROPE_BASE = 10000.0
SCALE = float(DH) ** -0.5

# per-core shards
NCORES = 8
HL = H // NCORES               # 4 local heads
DL = HL * DH                   # 512 local head dims
FL = F // NCORES               # 1024 local ffn
VL = V // NCORES               # 1024 local vocab
C = D // 128                   # 32 contraction tiles
NTT = T // 128                 # 12 key tiles
NMT = M // 128                 # 4 memory key tiles
NST = S // 128                 # 8 query tiles
NCH = 2                        # sequence chunks (512 each)
SC = S // NCH                  # 512

dt = mybir.dt
AF = mybir.ActivationFunctionType
ALU = mybir.AluOpType

NEG = -60000.0


def build_module():
    nc = bacc.Bacc("TRN2", target_bir_lowering=False, debug=False,
                   num_devices=NCORES)

    # const APs for activation scale/bias floats
    for v in (EPS, SCALE, 1.0 / D):
        t = nc.alloc_sbuf_tensor(f"cst_{v}", [128, 1], dt.float32)
        nc.gpsimd.memset(t.ap(), v)
        nc.const_aps.aps[(dt.float32, v)] = t.ap()

    # ---- kernel I/O ----
    h0T = nc.dram_tensor("h0T", [D, S], dt.float32, kind="ExternalInput")
    memT = nc.dram_tensor("memT", [L, D, M], dt.float16, kind="ExternalInput")
    wqkvT = nc.dram_tensor("wqkvT", [L, 3, D, DL], dt.float16, kind="ExternalInput")
    wmT = nc.dram_tensor("wmT", [L, 2, D, DL], dt.float16, kind="ExternalInput")
    woT = nc.dram_tensor("woT", [L, DL, D], dt.float16, kind="ExternalInput")
    wguT = nc.dram_tensor("wguT", [L, 2, D, FL], dt.float16, kind="ExternalInput")
    wdT = nc.dram_tensor("wdT", [L, FL, D], dt.float16, kind="ExternalInput")
    lmT = nc.dram_tensor("lmT", [D, VL], dt.float16, kind="ExternalInput")
    qcs = nc.dram_tensor("qcs", [2, 128, S], dt.float16, kind="ExternalInput")
    kcs = nc.dram_tensor("kcs", [2, 128, T], dt.float16, kind="ExternalInput")
    rmat_i = nc.dram_tensor("rmat", [128, 128], dt.float16, kind="ExternalInput")
    tmask = nc.dram_tensor("tmask", [128, 896], dt.float16, kind="ExternalInput")
    lnw = nc.dram_tensor("lnw", [5, 128, C], dt.float32, kind="ExternalInput")
    logitsT = nc.dram_tensor("logitsT", [VL, S], dt.float32, kind="ExternalOutput")

    with tile.TileContext(nc) as tc:
        with tc.tile_pool(name="sb", bufs=1) as sb, \
             tc.tile_pool(name="ps", bufs=1, space="PSUM") as ps, \
             tc.tile_pool(name="dr", bufs=1, space="DRAM") as dr:

            # ---- internal DRAM ----
            hdr = [dr.tile([D, S], dt.float32, tag=f"h{i}", bufs=1, name=f"h{i}")
                   for i in range(3)]           # h after resid 1..3
            arin = [[dr.tile([D, SC], dt.float32, tag=f"ai{i}{ch}", bufs=1,
                             name=f"ai{i}{ch}") for ch in range(NCH)]
                    for i in range(2 * L)]
            arout = [[dr.tile([D, SC], dt.float32, tag=f"ao{i}{ch}", bufs=1,
                              addr_space="Shared", name=f"ao{i}{ch}")
                      for ch in range(NCH)] for i in range(2 * L)]
            mkTd = [dr.tile([128, HL, M], dt.float16, tag=f"mk{l}", bufs=1,
                            name=f"mk{l}") for l in range(L)]
            mvd = [dr.tile([128, HL, NMT, DH], dt.float16, tag=f"mv{l}", bufs=1,
                           name=f"mv{l}") for l in range(L)]

            # ---- global constants in SBUF ----
            qc = sb.tile([128, 2, S], dt.float16, tag="qc", bufs=1, name="qc")
            nc.sync.dma_start(qc[:], qcs[:].rearrange("a p s -> p a s"))
            kc = sb.tile([128, 2, T], dt.float16, tag="kc", bufs=1, name="kc")
            nc.sync.dma_start(kc[:], kcs[:].rearrange("a p s -> p a s"))
            rmat = sb.tile([128, 128], dt.float16, tag="rm", bufs=1, name="rmat")
            nc.sync.dma_start(rmat[:], rmat_i[:])
            mask = sb.tile([128, 896], dt.float16, tag="msk", bufs=1, name="mask")
            nc.sync.dma_start(mask[:], tmask[:])
            lns = sb.tile([128, 5, C], dt.float32, tag="ln", bufs=1, name="lns")
            nc.sync.dma_start(lns[:], lnw[:].rearrange("a p c -> p a c"))
            ones_bf = sb.tile([128, 1], dt.float16, tag="o1", bufs=1, name="ones_bf")
            nc.vector.memset(ones_bf[:], 1.0)
            ones_row = sb.tile([1, 128], dt.float16, tag="o2", bufs=1, name="ones_row")
            nc.vector.memset(ones_row[:], 1.0)
            inv64_row = sb.tile([1, 128], dt.float16, tag="o3", bufs=1, name="inv64_row")
            nc.vector.memset(inv64_row[:], 1.0 / 64.0)

            def mm_ps(name):
                return ps.tile([128, 512], dt.float32, tag="mm", bufs=4, name=name)

            def aux_ps(name):
                return ps.tile([1, 512], dt.float32, tag="aux", bufs=2, name=name)

            def evf(name):
                return sb.tile([128, 512], dt.float32, tag="evf", bufs=2, name=name)

            def rope_apply(raw_ps, cos_ap, sin_ap, out_ap):
                """raw_ps: [128,512] psum f32 (pre-rope head tile, d on part).
                Writes rope'd bf16 into out_ap."""
                raw_bf = sb.tile([128, 512], dt.float16, tag="rraw", bufs=2,
                                 name="raw_bf")
                nc.vector.tensor_copy(raw_bf[:], raw_ps[:])
                r_ps = mm_ps("r_ps")
                nc.tensor.matmul(r_ps[:], rmat[:], raw_bf[:], start=True, stop=True)
                m1 = sb.tile([128, 512], dt.float16, tag="rt", bufs=2, name="m1")
                nc.vector.tensor_tensor(m1[:], raw_bf[:], cos_ap, ALU.mult)
                m2 = sb.tile([128, 512], dt.float16, tag="rt2", bufs=2, name="m2")
                nc.vector.tensor_tensor(m2[:], r_ps[:], sin_ap, ALU.mult)
                nc.vector.tensor_tensor(out_ap, m1[:], m2[:], ALU.add)

            # =========================================================
            # pre-phase: memory K/V projections for both layers -> DRAM
            # =========================================================
            for l in range(L):
                mem_sb = sb.tile([128, C, M], dt.float16, tag="xb", bufs=2,
                                 name=f"mem{l}")
                nc.sync.dma_start(mem_sb[:],
                                  memT[l].rearrange("(c p) m -> p c m", p=128))
                # mk: for each local head tile d -> [128, 512] then rope
                for half in range(2):
                    wmk = sb.tile([128, C, 256], dt.float16, tag="wp", bufs=2,
                                  name=f"wmk{l}{half}")
                    nc.sync.dma_start(
                        wmk[:],
                        wmT[l, 0, :, 256 * half:256 * (half + 1)]
                        .rearrange("(c p) n -> p c n", p=128))
                    for dd in range(2):
                        d = 2 * half + dd
                        acc = mm_ps(f"mk{l}{d}")
                        for c in range(C):
                            nc.tensor.matmul(acc[:], wmk[:, c, 128 * dd:128 * (dd + 1)],
                                             mem_sb[:, c, :], start=(c == 0),
                                             stop=(c == C - 1))
                        mko = sb.tile([128, 512], dt.float16, tag="pt", bufs=3,
                                      name="mko")
                        rope_apply(acc, kc[:, 0, :M], kc[:, 1, :M], mko[:])
                        nc.sync.dma_start(mkTd[l][:, d, :], mko[:])
                # mv: natural layout [m, d]
                for half in range(2):
                    wmv = sb.tile([128, C, 256], dt.float16, tag="wp", bufs=2,
                                  name=f"wmv{l}{half}")
                    nc.sync.dma_start(
                        wmv[:],
                        wmT[l, 1, :, 256 * half:256 * (half + 1)]
                        .rearrange("(c p) n -> p c n", p=128))
                    for mt in range(NMT):
                        acc = mm_ps(f"mv{l}{half}{mt}")
                        for c in range(C):
                            nc.tensor.matmul(acc[:, :256],
                                             mem_sb[:, c, 128 * mt:128 * (mt + 1)],
                                             wmv[:, c, :], start=(c == 0),
                                             stop=(c == C - 1))
                        mvo = sb.tile([128, 2, 128], dt.float16, tag="mvo", bufs=3,
                                      name="mvo")
                        nc.vector.tensor_copy(
                            mvo[:], acc[:, :256].rearrange("p (h d) -> p h d", d=128))
                        nc.sync.dma_start(mvd[l][:, 2 * half:2 * half + 2, mt, :],
                                          mvo[:])

            # =========================================================
            # rms pass: h_new = h_src (+ delta); write h_dst; xT = rms
            # =========================================================
            def rms_pass(h_src, delta, h_dst, ln_idx, xbufs, name):
                """h_src: DRAM [D, S] f32 AP; delta: list per chunk of DRAM
                [D, SC] or None; h_dst same form or None; xbufs: list per chunk
                of SBUF tiles [128, C, SC] bf16 (written in place)."""
                hv = h_src.rearrange("(c p) s -> p c s", p=128)
                for ch in range(NCH):
                    xb = xbufs[ch]
                    ssq = aux_ps(f"ssq_{name}{ch}")
                    for c in range(C):
                        ht = sb.tile([128, 512], dt.float32, tag="hl", bufs=2,
                                     name="ht")
                        nc.sync.dma_start(ht[:], hv[:, c, SC * ch:SC * (ch + 1)])
                        if delta is not None:
                            dtl = sb.tile([128, 512], dt.float32, tag="dl", bufs=2,
                                          name="dtl")
                            nc.sync.dma_start(
                                dtl[:],
                                delta[ch].rearrange("(c p) s -> p c s", p=128)[:, c, :])
                            hn = sb.tile([128, 512], dt.float32, tag="hn", bufs=2,
                                         name="hn")
                            nc.vector.tensor_tensor(hn[:], ht[:], dtl[:], ALU.add)
                            if h_dst is not None:
                                nc.sync.dma_start(
                                    h_dst.rearrange("(c p) s -> p c s", p=128)
                                    [:, c, SC * ch:SC * (ch + 1)], hn[:])
                        else:
                            hn = ht
                        # bf16 copy for pass 2 (in xb), square for sumsq
                        nc.vector.tensor_copy(xb[:, c, :], hn[:])
                        hsq = sb.tile([128, 512], dt.float16, tag="hsq", bufs=2,
                                      name="hsq")
                        nc.vector.tensor_tensor(hsq[:], hn[:], hn[:], ALU.mult)
                        nc.tensor.matmul(ssq[:], ones_bf[:], hsq[:],
                                         start=(c == 0), stop=(c == C - 1))
                    # rsqrt row and broadcast
                    sq = sb.tile([1, 512], dt.float32, tag="row", bufs=2, name="sq")
                    nc.scalar.activation(sq[:], ssq[:], AF.Sqrt, bias=EPS,
                                         scale=1.0 / D)
                    rs = sb.tile([1, 512], dt.float16, tag="row2", bufs=2, name="rs")
                    with nc.allow_low_precision(reason="fp16 row for broadcast mm"):
                        nc.vector.reciprocal(rs[:], sq[:])
                    bc = ps.tile([128, 512], dt.float32, tag="bc", bufs=2, name="bc")
                    nc.tensor.matmul(bc[:], ones_row[:], rs[:], start=True,
                                     stop=True)
                    for c in range(C):
                        nc.vector.scalar_tensor_tensor(
                            xb[:, c, :], xb[:, c, :], lns[:, ln_idx, c:c + 1],
                            bc[:], ALU.mult, ALU.mult)

            # =========================================================
            # attention + Wo for one layer; xbufs hold xT
            # =========================================================
            def attn_phase(l, xbufs, ar_site):
                # KT per head / V built first (k, v, then per-head q + attn)
                KT = sb.tile([128, HL, T], dt.float16, tag="KT", bufs=1,
                             name=f"KT{l}")
                Vt = sb.tile([128, HL, NTT, DH], dt.float16, tag="V", bufs=1,
                             name=f"V{l}")
                nc.sync.dma_start(KT[:, :, :M], mkTd[l][:])
                nc.sync.dma_start(Vt[:, :, :NMT, :], mvd[l][:])
                # k projection (rope'd) into KT self part
                for half in range(2):
                    wk = sb.tile([128, C, 256], dt.float16, tag="wp", bufs=2,
                                 name=f"wk{l}{half}")
                    nc.sync.dma_start(
                        wk[:], wqkvT[l, 1, :, 256 * half:256 * (half + 1)]
                        .rearrange("(c p) n -> p c n", p=128))
                    for dd in range(2):
                        d = 2 * half + dd
                        for ch in range(NCH):
                            acc = mm_ps(f"k{l}{d}{ch}")
                            for c in range(C):
                                nc.tensor.matmul(
                                    acc[:], wk[:, c, 128 * dd:128 * (dd + 1)],
                                    xbufs[ch][:, c, :], start=(c == 0),
                                    stop=(c == C - 1))
                            rope_apply(acc, kc[:, 0, M + SC * ch:M + SC * (ch + 1)],
                                       kc[:, 1, M + SC * ch:M + SC * (ch + 1)],
                                       KT[:, d, M + SC * ch:M + SC * (ch + 1)])
                # v projection (natural layout)
                for half in range(2):
                    wv = sb.tile([128, C, 256], dt.float16, tag="wp", bufs=2,
                                 name=f"wv{l}{half}")
                    nc.sync.dma_start(
                        wv[:], wqkvT[l, 2, :, 256 * half:256 * (half + 1)]
                        .rearrange("(c p) n -> p c n", p=128))
                    for st in range(NST):
                        ch, sti = st // 4, st % 4
                        acc = mm_ps(f"v{l}{half}{st}")
                        for c in range(C):
                            nc.tensor.matmul(
                                acc[:, :256],
                                xbufs[ch][:, c, 128 * sti:128 * (sti + 1)],
                                wv[:, c, :], start=(c == 0), stop=(c == C - 1))
                        nc.vector.tensor_copy(
                            Vt[:, 2 * half:2 * half + 2, NMT + st, :],
                            acc[:, :256].rearrange("p (h d) -> p h d", d=128))
                # per-head: q proj + attention
                oT = sb.tile([128, HL, S], dt.float16, tag="oT", bufs=1,
                             name=f"oT{l}")
                for half in range(2):
                    wqh = sb.tile([128, C, 256], dt.float16, tag="wp", bufs=2,
                                  name=f"wq{l}{half}")
                    nc.sync.dma_start(
                        wqh[:], wqkvT[l, 0, :, 256 * half:256 * (half + 1)]
                        .rearrange("(c p) n -> p c n", p=128))
                    for hh in range(2):
                        h = 2 * half + hh
                        qT = sb.tile([128, S], dt.float16, tag="qT", bufs=2,
                                     name=f"qT{l}{h}")
                        for ch in range(NCH):
                            acc = mm_ps(f"q{l}{h}{ch}")
                            for c in range(C):
                                nc.tensor.matmul(
                                    acc[:], wqh[:, c, 128 * hh:128 * (hh + 1)],
                                    xbufs[ch][:, c, :], start=(c == 0),
                                    stop=(c == C - 1))
                            rope_apply(acc, qc[:, 0, SC * ch:SC * (ch + 1)],
                                       qc[:, 1, SC * ch:SC * (ch + 1)],
                                       qT[:, SC * ch:SC * (ch + 1)])
                        for sb_i in range(NCH):
                            ntt = NMT + 4 * (sb_i + 1)
                            o_ps = mm_ps(f"o{l}{h}{sb_i}")
                            s_ps = aux_ps(f"s{l}{h}{sb_i}")
                            for tt in range(ntt):
                                sc_ps = mm_ps(f"sc{l}{h}{sb_i}{tt}")
                                nc.tensor.matmul(sc_ps[:],
                                                 KT[:, h, 128 * tt:128 * (tt + 1)],
                                                 qT[:, SC * sb_i:SC * (sb_i + 1)],
                                                 start=True, stop=True)
                                dtile = tt - ntt + 4      # >= 0 -> diagonal tile
                                if dtile >= 0:
                                    off = 384 - 128 * dtile
                                    nc.vector.tensor_tensor(
                                        sc_ps[:], sc_ps[:],
                                        mask[:, off:off + 512], ALU.add)
                                pt = sb.tile([128, 512], dt.float16, tag="pt",
                                             bufs=3, name="pt")
                                nc.scalar.activation(pt[:], sc_ps[:], AF.Exp,
                                                     scale=SCALE)
                                nc.tensor.matmul(o_ps[:], Vt[:, h, tt, :], pt[:],
                                                 start=(tt == 0),
                                                 stop=(tt == ntt - 1))
                                nc.tensor.matmul(s_ps[:], ones_bf[:], pt[:],
                                                 start=(tt == 0),
                                                 stop=(tt == ntt - 1))
                            rrf = sb.tile([1, 512], dt.float32, tag="rowf", bufs=2,
                                          name="rrf")
                            nc.vector.reciprocal(rrf[:], s_ps[:])
                            rr = sb.tile([1, 512], dt.float16, tag="row2", bufs=2,
                                         name="rr")
                            with nc.allow_low_precision(reason="fp16 row for broadcast mm"):
                                nc.vector.tensor_scalar_mul(rr[:], rrf[:], 64.0)
                            bc = ps.tile([128, 512], dt.float32, tag="bc", bufs=2,
                                         name="bca")
                            nc.tensor.matmul(bc[:], inv64_row[:], rr[:],
                                             start=True, stop=True)
                            bcs = sb.tile([128, 512], dt.float32, tag="bcs",
                                          bufs=2, name="bcs")
                            nc.vector.tensor_copy(bcs[:], bc[:])
                            nc.vector.tensor_tensor(
                                oT[:, h, SC * sb_i:SC * (sb_i + 1)],
                                o_ps[:], bcs[:], ALU.mult)
                # Wo: out [Do, s] partial sums -> arin
                for half in range(2):
                    wo = sb.tile([128, HL, 2048], dt.float16, tag="wp", bufs=2,
                                 name=f"wo{l}{half}")
                    nc.sync.dma_start(
                        wo[:], woT[l, :, 2048 * half:2048 * (half + 1)]
                        .rearrange("(h p) n -> p h n", p=128))
                    for do in range(16):
                        for ch in range(NCH):
                            acc = mm_ps(f"wo{l}{half}{do}{ch}")
                            for hh in range(HL):
                                nc.tensor.matmul(
                                    acc[:], wo[:, hh, 128 * do:128 * (do + 1)],
                                    oT[:, hh, SC * ch:SC * (ch + 1)],
                                    start=(hh == 0), stop=(hh == HL - 1))
                            ev = evf("woev")
                            nc.vector.tensor_copy(ev[:], acc[:])
                            nc.sync.dma_start(
                                arin[ar_site][ch]
                                .rearrange("(t p) s -> p t s", p=128)
                                [:, 16 * half + do, :], ev[:])
                for ch in range(NCH):
                    nc.gpsimd.collective_compute(
                        "AllReduce", ALU.add,
                        replica_groups=[list(range(NCORES))],
                        ins=[arin[ar_site][ch][:]], outs=[arout[ar_site][ch][:]])

            # =========================================================
            # FFN for one layer: xbufs -> partial down-proj -> arin
            # =========================================================
            def ffn_phase(l, xbufs, ar_site):
                actT = sb.tile([128, FL // 128, S], dt.float16, tag="actT",
                               bufs=1, name=f"actT{l}")
                for fe in range(FL // 128):
                    wg = sb.tile([128, C, 128], dt.float16, tag="wp", bufs=2,
                                 name=f"wg{l}{fe}")
                    nc.sync.dma_start(
                        wg[:], wguT[l, 0, :, 128 * fe:128 * (fe + 1)]
                        .rearrange("(c p) n -> p c n", p=128))
                    gs = sb.tile([128, S], dt.float16, tag="gs", bufs=2,
                                 name="gs")
                    for ch in range(NCH):
                        acc = mm_ps(f"g{l}{fe}{ch}")
                        for c in range(C):
                            nc.tensor.matmul(acc[:], wg[:, c, :],
                                             xbufs[ch][:, c, :], start=(c == 0),
                                             stop=(c == C - 1))
                        nc.scalar.activation(gs[:, SC * ch:SC * (ch + 1)], acc[:],
                                             AF.Silu)
                    wu = sb.tile([128, C, 128], dt.float16, tag="wp", bufs=2,
                                 name=f"wu{l}{fe}")
                    nc.sync.dma_start(
                        wu[:], wguT[l, 1, :, 128 * fe:128 * (fe + 1)]
                        .rearrange("(c p) n -> p c n", p=128))
                    for ch in range(NCH):
                        acc = mm_ps(f"u{l}{fe}{ch}")
                        for c in range(C):
                            nc.tensor.matmul(acc[:], wu[:, c, :],
                                             xbufs[ch][:, c, :], start=(c == 0),
                                             stop=(c == C - 1))
                        nc.vector.tensor_tensor(
                            actT[:, fe, SC * ch:SC * (ch + 1)], acc[:],
                            gs[:, SC * ch:SC * (ch + 1)], ALU.mult)
                # down proj
                for quarter in range(4):
                    wd = sb.tile([128, FL // 128, 1024], dt.float16, tag="wp",
                                 bufs=2, name=f"wd{l}{quarter}")
                    nc.sync.dma_start(
                        wd[:], wdT[l, :, 1024 * quarter:1024 * (quarter + 1)]
                        .rearrange("(f p) n -> p f n", p=128))
                    for do in range(8):
                        for ch in range(NCH):
                            acc = mm_ps(f"wd{l}{quarter}{do}{ch}")
                            for fc in range(FL // 128):
                                nc.tensor.matmul(
                                    acc[:], wd[:, fc, 128 * do:128 * (do + 1)],
                                    actT[:, fc, SC * ch:SC * (ch + 1)],
                                    start=(fc == 0), stop=(fc == FL // 128 - 1))
                            ev = evf("wdev")
                            nc.vector.tensor_copy(ev[:], acc[:])
                            nc.sync.dma_start(
                                arin[ar_site][ch]
                                .rearrange("(t p) s -> p t s", p=128)
                                [:, 8 * quarter + do, :], ev[:])
                for ch in range(NCH):
                    nc.gpsimd.collective_compute(
                        "AllReduce", ALU.add,
                        replica_groups=[list(range(NCORES))],
                        ins=[arin[ar_site][ch][:]], outs=[arout[ar_site][ch][:]])

            # =========================================================
            # main flow
            # =========================================================
            def xb_tiles(nm):
                return [sb.tile([128, C, SC], dt.float16, tag="xb", bufs=2,
                                name=f"{nm}{ch}") for ch in range(NCH)]

            # layer 0
            x0 = xb_tiles("x0")
            rms_pass(h0T[:], None, None, 0, x0, "r0")
            attn_phase(0, x0, 0)
            x1 = xb_tiles("x1")
            rms_pass(h0T[:], arout[0], hdr[0][:], 1, x1, "r1")
            ffn_phase(0, x1, 1)
            # layer 1
            x2 = xb_tiles("x2")
            rms_pass(hdr[0][:], arout[1], hdr[1][:], 2, x2, "r2")
            attn_phase(1, x2, 2)
            x3 = xb_tiles("x3")
            rms_pass(hdr[1][:], arout[2], hdr[2][:], 3, x3, "r3")
            ffn_phase(1, x3, 3)
            # final rms + lm head
            xf = xb_tiles("xf")
            rms_pass(hdr[2][:], arout[3], None, 4, xf, "rf")
            for vq in range(4):
                lm = sb.tile([128, C, 256], dt.float16, tag="wp", bufs=2,
                             name=f"lm{vq}")
                nc.sync.dma_start(lm[:], lmT[:, 256 * vq:256 * (vq + 1)]
                                  .rearrange("(c p) n -> p c n", p=128))
                for vv in range(2):
                    for ch in range(NCH):
                        acc = mm_ps(f"lm{vq}{vv}{ch}")
                        for c in range(C):
                            nc.tensor.matmul(acc[:], lm[:, c, 128 * vv:128 * (vv + 1)],
                                             xf[ch][:, c, :], start=(c == 0),
                                             stop=(c == C - 1))
                        ev = evf("lmev")
                        nc.vector.tensor_copy(ev[:], acc[:])
                        nc.sync.dma_start(
                            logitsT[:].rearrange("(t p) s -> p t s", p=128)
                            [:, 2 * vq + vv, SC * ch:SC * (ch + 1)], ev[:])

    nc.finalize()
    return nc


_NC_CACHE = {}


def _get_module():
    if "nc" not in _NC_CACHE:
        _NC_CACHE["nc"] = build_module()
    return _NC_CACHE["nc"]


def _rope_tables():
    inv_freq = 1.0 / (ROPE_BASE ** (np.arange(0, DH, 2, dtype=np.float64) / DH))
    ang = np.arange(T, dtype=np.float64)[:, None] * inv_freq[None, :]
    emb = np.concatenate([ang, ang], axis=-1)          # [T, DH]
    return np.cos(emb).astype(np.float32), np.sin(emb).astype(np.float32)


def kernel(input_ids, memory, embed, Wq, Wk, Wv, Wo, Wg, Wu, Wd, Wmk, Wmv,
           ln1, ln2, normw, lm_head):
    input_ids = np.asarray(input_ids)
    f32 = np.float32
    memory = np.asarray(memory, f32)

    nc = _get_module()

    # host prep: embedding gather (pure data movement) + layout transforms
    h0 = np.asarray(embed, f32)[input_ids.reshape(-1)]          # [S, D]
    h0T = np.ascontiguousarray(h0.T)                            # [D, S] f32

    cos, sin = _rope_tables()
    qcs = np.stack([cos[M:], sin[M:]]).transpose(0, 2, 1)       # [2, 128, S]
    kcs = np.stack([cos, sin]).transpose(0, 2, 1)               # [2, 128, T]

    rmat = np.zeros((128, 128), f32)
    for d in range(64):
        rmat[d + 64, d] = -1.0
        rmat[d, d + 64] = 1.0

    tmaskv = np.full((128, 896), NEG, f32)
    for t in range(128):
        tmaskv[t, 384 + t:] = 0.0

    def bf(x):
        return np.ascontiguousarray(x).astype(BF)

    memT = np.stack([memory[l, 0].T for l in range(L)])         # [L, D, M]

    in_maps = []
    for i in range(NCORES):
        hs = slice(DL * i, DL * (i + 1))
        fs = slice(FL * i, FL * (i + 1))
        vs = slice(VL * i, VL * (i + 1))
        lnw = np.stack([np.asarray(ln1, f32)[0], np.asarray(ln2, f32)[0],
                        np.asarray(ln1, f32)[1], np.asarray(ln2, f32)[1],
                        np.asarray(normw, f32)])                # [5, D]
        in_maps.append({
            "h0T": h0T,
            "memT": bf(memT),
            "wqkvT": bf(np.stack([np.stack([np.asarray(W, f32)[l][hs].T
                                            for W in (Wq, Wk, Wv)])
                                  for l in range(L)])),
            "wmT": bf(np.stack([np.stack([np.asarray(W, f32)[l][hs].T
                                          for W in (Wmk, Wmv)])
                                for l in range(L)])),
            "woT": bf(np.stack([np.asarray(Wo, f32)[l][:, hs].T
                                for l in range(L)])),
            "wguT": bf(np.stack([np.stack([np.asarray(W, f32)[l][fs].T
                                           for W in (Wg, Wu)])
                                 for l in range(L)])),
            "wdT": bf(np.stack([np.asarray(Wd, f32)[l][:, fs].T
                                for l in range(L)])),
            "lmT": bf(np.asarray(lm_head, f32)[vs].T),
            "qcs": bf(qcs),
            "kcs": bf(kcs),
            "rmat": bf(rmat),
            "tmask": bf(tmaskv),
            "lnw": np.ascontiguousarray(
                lnw.reshape(5, C, 128).transpose(0, 2, 1)),     # [5, 128, C]
        })

    res = run_bass_kernel_spmd(nc, in_maps, core_ids=list(range(NCORES)))
    _NC_CACHE["last_results"] = res

    logits = np.empty((B, S, V), f32)
    for i in range(NCORES):
        logits[0, :, VL * i:VL * (i + 1)] = res.results[i]["logitsT"].T
    return logits


# revision 19
# speedup vs baseline: 1.1281x; 1.1281x over previous
"""Trainium2 Bass kernel for a 2-layer LLaMA-style decoder with per-layer
memory K/V prefix (tokenmix2 Decoder), tensor-parallel over 8 NeuronCores.

Sharding: heads (32 -> 4/core), FFN intermediate (8192 -> 1024/core),
vocab (8192 -> 1024/core).  Two AllReduces per layer (attention out,
FFN out), each split into two sequence-chunk collectives for overlap.

Layouts: activations are kept transposed (xT [D, S]) so every matmul
output feeds the next without transposes.  Attention computes
scoresT [t, s] per head; softmax runs without max-subtraction (scores
are ~N(0,1) after the 1/sqrt(Dh) scale) with the normalization applied
on the oT psum evacuation.  Matmul compute in bf16, residual stream and
psum accumulation in fp32.
"""
import sys

sys.path.insert(0, "/opt/trn_rl_repo")

import numpy as np
import ml_dtypes

import concourse.bass as bass
import concourse.mybir as mybir
import concourse.tile as tile
from concourse import bacc
from concourse.bass_utils import run_bass_kernel_spmd

BF = np.float16

# model dims
L, D, H, DH, F, V = 2, 4096, 32, 128, 8192, 8192
B, S, M = 1, 1024, 512
T = M + S                      # 1536 total key positions
EPS = 1e-5
ROPE_BASE = 10000.0
SCALE = float(DH) ** -0.5

# per-core shards
NCORES = 8
HL = H // NCORES               # 4 local heads
DL = HL * DH                   # 512 local head dims
FL = F // NCORES               # 1024 local ffn
VL = V // NCORES               # 1024 local vocab
C = D // 128                   # 32 contraction tiles
NTT = T // 128                 # 12 key tiles
NMT = M // 128                 # 4 memory key tiles
NST = S // 128                 # 8 query tiles
NCH = 2                        # sequence chunks (512 each)
SC = S // NCH                  # 512

dt = mybir.dt
AF = mybir.ActivationFunctionType
ALU = mybir.AluOpType

NEG = -60000.0


def build_module():
    nc = bacc.Bacc("TRN2", target_bir_lowering=False, debug=False,
                   num_devices=NCORES)

    # const APs for activation scale/bias floats
    for v in (EPS, SCALE, 1.0 / D):
        t = nc.alloc_sbuf_tensor(f"cst_{v}", [128, 1], dt.float32)
        nc.gpsimd.memset(t.ap(), v)
        nc.const_aps.aps[(dt.float32, v)] = t.ap()

    # ---- kernel I/O ----
    h0T = nc.dram_tensor("h0T", [D, S], dt.float32, kind="ExternalInput")
    memT = nc.dram_tensor("memT", [L, 128, C, M], dt.float16, kind="ExternalInput")
    wqkvT = nc.dram_tensor("wqkvT", [L, 3, 2, 128, C, 256], dt.float16, kind="ExternalInput")
    wmT = nc.dram_tensor("wmT", [L, 2, 2, 128, C, 256], dt.float16, kind="ExternalInput")
    woT = nc.dram_tensor("woT", [L, 2, 128, HL, 2048], dt.float16, kind="ExternalInput")
    wguT = nc.dram_tensor("wguT", [L, 2, 8, 128, C, 128], dt.float16, kind="ExternalInput")
    wdT = nc.dram_tensor("wdT", [L, 4, 128, 8, 1024], dt.float16, kind="ExternalInput")
    lmT = nc.dram_tensor("lmT", [4, 128, C, 256], dt.float16, kind="ExternalInput")
    qcs = nc.dram_tensor("qcs", [128, 2, S], dt.float16, kind="ExternalInput")
    kcs = nc.dram_tensor("kcs", [128, 2, T], dt.float16, kind="ExternalInput")
    rmat_i = nc.dram_tensor("rmat", [128, 128], dt.float16, kind="ExternalInput")
    tmask = nc.dram_tensor("tmask", [128, 896], dt.float16, kind="ExternalInput")
    lnw = nc.dram_tensor("lnw", [128, 5, C], dt.float32, kind="ExternalInput")
    logitsT = nc.dram_tensor("logitsT", [VL, S], dt.float32, kind="ExternalOutput")

    with tile.TileContext(nc) as tc:
        with tc.tile_pool(name="sb", bufs=1) as sb, \
             tc.tile_pool(name="ps", bufs=1, space="PSUM") as ps, \
             tc.tile_pool(name="dr", bufs=1, space="DRAM") as dr:

            # ---- internal DRAM ----
            hdr = [dr.tile([D, S], dt.float32, tag=f"h{i}", bufs=1, name=f"h{i}")
                   for i in range(3)]           # h after resid 1..3
            arin = [[dr.tile([D, SC], dt.float16, tag=f"ai{i}{ch}", bufs=1,
                             name=f"ai{i}{ch}") for ch in range(NCH)]
                    for i in range(2 * L)]
            arout = [[dr.tile([D, SC], dt.float16, tag=f"ao{i}{ch}", bufs=1,
                              addr_space="Shared", name=f"ao{i}{ch}")
                      for ch in range(NCH)] for i in range(2 * L)]
            mkTd = [dr.tile([128, HL, M], dt.float16, tag=f"mk{l}", bufs=1,
                            name=f"mk{l}") for l in range(L)]
            mvd = [dr.tile([128, HL, NMT, DH], dt.float16, tag=f"mv{l}", bufs=1,
                           name=f"mv{l}") for l in range(L)]

            # ---- global constants in SBUF ----
            qc = sb.tile([128, 2, S], dt.float16, tag="qc", bufs=1, name="qc")
            nc.sync.dma_start(qc[:], qcs[:])
            kc = sb.tile([128, 2, T], dt.float16, tag="kc", bufs=1, name="kc")
            nc.sync.dma_start(kc[:], kcs[:])
            rmat = sb.tile([128, 128], dt.float16, tag="rm", bufs=1, name="rmat")
            nc.sync.dma_start(rmat[:], rmat_i[:])
            mask = sb.tile([128, 896], dt.float16, tag="msk", bufs=1, name="mask")
            nc.sync.dma_start(mask[:], tmask[:])
            lns = sb.tile([128, 5, C], dt.float32, tag="ln", bufs=1, name="lns")
            nc.sync.dma_start(lns[:], lnw[:])
            ones_bf = sb.tile([128, 1], dt.float16, tag="o1", bufs=1, name="ones_bf")
            nc.vector.memset(ones_bf[:], 1.0)
            ones_row = sb.tile([1, 128], dt.float16, tag="o2", bufs=1, name="ones_row")
            nc.vector.memset(ones_row[:], 1.0)
            inv64_row = sb.tile([1, 128], dt.float16, tag="o3", bufs=1, name="inv64_row")
            nc.vector.memset(inv64_row[:], 1.0 / 64.0)

            def mm_ps(name):
                return ps.tile([128, 512], dt.float32, tag="mm", bufs=4, name=name)

            def aux_ps(name):
                return ps.tile([1, 512], dt.float32, tag="aux", bufs=2, name=name)

            def evf(name):
                return sb.tile([128, 512], dt.float32, tag="evf", bufs=2, name=name)

            def evh(name):
                return sb.tile([128, 512], dt.float16, tag="evh", bufs=2, name=name)

            def rope_apply(raw_ps, cos_ap, sin_ap, out_ap):
                """raw_ps: [128,512] psum f32 (pre-rope head tile, d on part).
                Writes rope'd bf16 into out_ap."""
                raw_bf = sb.tile([128, 512], dt.float16, tag="rraw", bufs=2,
                                 name="raw_bf")
                nc.vector.tensor_copy(raw_bf[:], raw_ps[:])
                r_ps = mm_ps("r_ps")
                nc.tensor.matmul(r_ps[:], rmat[:], raw_bf[:], start=True, stop=True)
                m1 = sb.tile([128, 512], dt.float16, tag="rt", bufs=2, name="m1")
                nc.vector.tensor_tensor(m1[:], raw_bf[:], cos_ap, ALU.mult)
                m2 = sb.tile([128, 512], dt.float16, tag="rt2", bufs=2, name="m2")
                nc.vector.tensor_tensor(m2[:], r_ps[:], sin_ap, ALU.mult)
                nc.vector.tensor_tensor(out_ap, m1[:], m2[:], ALU.add)

            # =========================================================
            # pre-phase: memory K/V projections for both layers -> DRAM
            # =========================================================
            for l in range(L):
                mem_sb = sb.tile([128, C, M], dt.float16, tag="xb", bufs=2,
                                 name=f"mem{l}")
                nc.sync.dma_start(mem_sb[:], memT[l])
                # mk: for each local head tile d -> [128, 512] then rope
                for half in range(2):
                    wmk = sb.tile([128, C, 256], dt.float16, tag="wp", bufs=2,
                                  name=f"wmk{l}{half}")
                    nc.sync.dma_start(wmk[:], wmT[l, 0, half])
                    for dd in range(2):
                        d = 2 * half + dd
                        acc = mm_ps(f"mk{l}{d}")
                        for c in range(C):
                            nc.tensor.matmul(acc[:], wmk[:, c, 128 * dd:128 * (dd + 1)],
                                             mem_sb[:, c, :], start=(c == 0),
                                             stop=(c == C - 1))
                        mko = sb.tile([128, 512], dt.float16, tag="pt", bufs=3,
                                      name="mko")
                        rope_apply(acc, kc[:, 0, :M], kc[:, 1, :M], mko[:])
                        nc.sync.dma_start(mkTd[l][:, d, :], mko[:])
                # mv: natural layout [m, d]
                for half in range(2):
                    wmv = sb.tile([128, C, 256], dt.float16, tag="wp", bufs=2,
                                  name=f"wmv{l}{half}")
                    nc.sync.dma_start(wmv[:], wmT[l, 1, half])
                    for mt in range(NMT):
                        acc = mm_ps(f"mv{l}{half}{mt}")
                        for c in range(C):
                            nc.tensor.matmul(acc[:, :256],
                                             mem_sb[:, c, 128 * mt:128 * (mt + 1)],
                                             wmv[:, c, :], start=(c == 0),
                                             stop=(c == C - 1))
                        mvo = sb.tile([128, 2, 128], dt.float16, tag="mvo", bufs=3,
                                      name="mvo")
                        nc.vector.tensor_copy(
                            mvo[:], acc[:, :256].rearrange("p (h d) -> p h d", d=128))
                        nc.sync.dma_start(mvd[l][:, 2 * half:2 * half + 2, mt, :],
                                          mvo[:])

            # =========================================================
            # rms pass: h_new = h_src (+ delta); write h_dst; xT = rms
            # =========================================================
            def rms_pass(h_src, delta, h_dst, ln_idx, xbufs, name):
                """h_src: DRAM [D, S] f32 AP; delta: list per chunk of DRAM
                [D, SC] or None; h_dst same form or None; xbufs: list per chunk
                of SBUF tiles [128, C, SC] bf16 (written in place)."""
                hv = h_src.rearrange("(c p) s -> p c s", p=128)
                for ch in range(NCH):
                    xb = xbufs[ch]
                    ssq = aux_ps(f"ssq_{name}{ch}")
                    for c in range(C):
                        ht = sb.tile([128, 512], dt.float32, tag="hl", bufs=2,
                                     name="ht")
                        nc.sync.dma_start(ht[:], hv[:, c, SC * ch:SC * (ch + 1)])
                        if delta is not None:
                            dtl = sb.tile([128, 512], dt.float16, tag="dl", bufs=2,
                                          name="dtl")
                            nc.sync.dma_start(
                                dtl[:],
                                delta[ch].rearrange("(c p) s -> p c s", p=128)[:, c, :])
                            hn = sb.tile([128, 512], dt.float32, tag="hn", bufs=2,
                                         name="hn")
                            nc.vector.tensor_tensor(hn[:], ht[:], dtl[:], ALU.add)
                            if h_dst is not None:
                                nc.sync.dma_start(
                                    h_dst.rearrange("(c p) s -> p c s", p=128)
                                    [:, c, SC * ch:SC * (ch + 1)], hn[:])
                        else:
                            hn = ht
                        # fp16 copy for pass 2 (in xb), square for sumsq
                        nc.gpsimd.tensor_copy(xb[:, c, :], hn[:])
                        hsq = sb.tile([128, 512], dt.float16, tag="hsq", bufs=2,
                                      name="hsq")
                        nc.gpsimd.tensor_mul(hsq[:], hn[:], hn[:])
                        nc.tensor.matmul(ssq[:], ones_bf[:], hsq[:],
                                         start=(c == 0), stop=(c == C - 1))
                    # rsqrt row and broadcast
                    sq = sb.tile([1, 512], dt.float32, tag="row", bufs=2, name="sq")
                    nc.scalar.activation(sq[:], ssq[:], AF.Sqrt, bias=EPS,
                                         scale=1.0 / D)
                    rs = sb.tile([1, 512], dt.float16, tag="row2", bufs=2, name="rs")
                    with nc.allow_low_precision(reason="fp16 row for broadcast mm"):
                        nc.vector.reciprocal(rs[:], sq[:])
                    bc = ps.tile([128, 512], dt.float32, tag="bc", bufs=2, name="bc")
                    nc.tensor.matmul(bc[:], ones_row[:], rs[:], start=True,
                                     stop=True)
                    for c in range(C):
                        nc.vector.scalar_tensor_tensor(
                            xb[:, c, :], xb[:, c, :], lns[:, ln_idx, c:c + 1],
                            bc[:], ALU.mult, ALU.mult)

            # =========================================================
            # attention + Wo for one layer; xbufs hold xT
            # =========================================================
            def attn_phase(l, xbufs, ar_site):
                # KT per head / V built first (k, v, then per-head q + attn)
                KT = sb.tile([128, HL, T], dt.float16, tag="KT", bufs=1,
                             name=f"KT{l}")
                Vt = sb.tile([128, HL, NTT, DH], dt.float16, tag="V", bufs=1,
                             name=f"V{l}")
                nc.sync.dma_start(KT[:, :, :M], mkTd[l][:])
                nc.sync.dma_start(Vt[:, :, :NMT, :], mvd[l][:])
                # k projection (rope'd) into KT self part
                for half in range(2):
                    wk = sb.tile([128, C, 256], dt.float16, tag="wp", bufs=2,
                                 name=f"wk{l}{half}")
                    nc.sync.dma_start(wk[:], wqkvT[l, 1, half])
                    for dd in range(2):
                        d = 2 * half + dd
                        for ch in range(NCH):
                            acc = mm_ps(f"k{l}{d}{ch}")
                            for c in range(C):
                                nc.tensor.matmul(
                                    acc[:], wk[:, c, 128 * dd:128 * (dd + 1)],
                                    xbufs[ch][:, c, :], start=(c == 0),
                                    stop=(c == C - 1))
                            rope_apply(acc, kc[:, 0, M + SC * ch:M + SC * (ch + 1)],
                                       kc[:, 1, M + SC * ch:M + SC * (ch + 1)],
                                       KT[:, d, M + SC * ch:M + SC * (ch + 1)])
                # v projection (natural layout)
                for half in range(2):
                    wv = sb.tile([128, C, 256], dt.float16, tag="wp", bufs=2,
                                 name=f"wv{l}{half}")
                    nc.sync.dma_start(wv[:], wqkvT[l, 2, half])
                    for st in range(NST):
                        ch, sti = st // 4, st % 4
                        acc = mm_ps(f"v{l}{half}{st}")
                        for c in range(C):
                            nc.tensor.matmul(
                                acc[:, :256],
                                xbufs[ch][:, c, 128 * sti:128 * (sti + 1)],
                                wv[:, c, :], start=(c == 0), stop=(c == C - 1))
                        nc.vector.tensor_copy(
                            Vt[:, 2 * half:2 * half + 2, NMT + st, :],
                            acc[:, :256].rearrange("p (h d) -> p h d", d=128))
                # per-head: q proj + attention
                oT = sb.tile([128, HL, S], dt.float16, tag="oT", bufs=1,
                             name=f"oT{l}")
                for half in range(2):
                    wqh = sb.tile([128, C, 256], dt.float16, tag="wp", bufs=2,
                                  name=f"wq{l}{half}")
                    nc.sync.dma_start(wqh[:], wqkvT[l, 0, half])
                    for hh in range(2):
                        h = 2 * half + hh
                        qT = sb.tile([128, S], dt.float16, tag="qT", bufs=2,
                                     name=f"qT{l}{h}")
                        for ch in range(NCH):
                            acc = mm_ps(f"q{l}{h}{ch}")
                            for c in range(C):
                                nc.tensor.matmul(
                                    acc[:], wqh[:, c, 128 * hh:128 * (hh + 1)],
                                    xbufs[ch][:, c, :], start=(c == 0),
                                    stop=(c == C - 1))
                            rope_apply(acc, qc[:, 0, SC * ch:SC * (ch + 1)],
                                       qc[:, 1, SC * ch:SC * (ch + 1)],
                                       qT[:, SC * ch:SC * (ch + 1)])
                        for sb_i in range(NCH):
                            ntt = NMT + 4 * (sb_i + 1)
                            o_ps = mm_ps(f"o{l}{h}{sb_i}")
                            s_ps = aux_ps(f"s{l}{h}{sb_i}")
                            for tt in range(ntt):
                                sc_ps = mm_ps(f"sc{l}{h}{sb_i}{tt}")
                                nc.tensor.matmul(sc_ps[:],
                                                 KT[:, h, 128 * tt:128 * (tt + 1)],
                                                 qT[:, SC * sb_i:SC * (sb_i + 1)],
                                                 start=True, stop=True)
                                dtile = tt - ntt + 4      # >= 0 -> diagonal tile
                                if dtile >= 0:
                                    off = 384 - 128 * dtile
                                    nc.vector.tensor_tensor(
                                        sc_ps[:], sc_ps[:],
                                        mask[:, off:off + 512], ALU.add)
                                pt = sb.tile([128, 512], dt.float16, tag="pt",
                                             bufs=3, name="pt")
                                nc.scalar.activation(pt[:], sc_ps[:], AF.Exp,
                                                     scale=SCALE)
                                nc.tensor.matmul(o_ps[:], Vt[:, h, tt, :], pt[:],
                                                 start=(tt == 0),
                                                 stop=(tt == ntt - 1))
                                nc.tensor.matmul(s_ps[:], ones_bf[:], pt[:],
                                                 start=(tt == 0),
                                                 stop=(tt == ntt - 1))
                            rrf = sb.tile([1, 512], dt.float32, tag="rowf", bufs=2,
                                          name="rrf")
                            nc.vector.reciprocal(rrf[:], s_ps[:])
                            rr = sb.tile([1, 512], dt.float16, tag="row2", bufs=2,
                                         name="rr")
                            with nc.allow_low_precision(reason="fp16 row for broadcast mm"):
                                nc.vector.tensor_scalar_mul(rr[:], rrf[:], 64.0)
                            bc = ps.tile([128, 512], dt.float32, tag="bc", bufs=2,
                                         name="bca")
                            nc.tensor.matmul(bc[:], inv64_row[:], rr[:],
                                             start=True, stop=True)
                            bcs = sb.tile([128, 512], dt.float32, tag="bcs",
                                          bufs=2, name="bcs")
                            nc.vector.tensor_copy(bcs[:], bc[:])
                            nc.vector.tensor_tensor(
                                oT[:, h, SC * sb_i:SC * (sb_i + 1)],
                                o_ps[:], bcs[:], ALU.mult)
                # Wo: out [Do, s] partial sums -> arin
                for ch in range(NCH):
                  for half in range(2):
                    wo = sb.tile([128, HL, 2048], dt.float16, tag="wp", bufs=2,
                                 name=f"wo{l}{ch}{half}")
                    nc.sync.dma_start(wo[:], woT[l, half])
                    for do in range(16):
                        if True:
                            acc = mm_ps(f"wo{l}{half}{do}{ch}")
                            for hh in range(HL):
                                nc.tensor.matmul(
                                    acc[:], wo[:, hh, 128 * do:128 * (do + 1)],
                                    oT[:, hh, SC * ch:SC * (ch + 1)],
                                    start=(hh == 0), stop=(hh == HL - 1))
                            ev = evh("woev")
                            nc.vector.tensor_copy(ev[:], acc[:])
                            nc.sync.dma_start(
                                arin[ar_site][ch]
                                .rearrange("(t p) s -> p t s", p=128)
                                [:, 16 * half + do, :], ev[:])
                  nc.gpsimd.collective_compute(
                      "AllReduce", ALU.add,
                      replica_groups=[list(range(NCORES))],
                      ins=[arin[ar_site][ch][:]], outs=[arout[ar_site][ch][:]])

            # =========================================================
            # FFN for one layer: xbufs -> partial down-proj -> arin
            # =========================================================
            def ffn_phase(l, xbufs, ar_site):
                actT = sb.tile([128, FL // 128, S], dt.float16, tag="actT",
                               bufs=1, name=f"actT{l}")
                for fe in range(FL // 128):
                    wg = sb.tile([128, C, 128], dt.float16, tag="wp", bufs=2,
                                 name=f"wg{l}{fe}")
                    nc.sync.dma_start(wg[:], wguT[l, 0, fe])
                    gs = sb.tile([128, S], dt.float16, tag="gs", bufs=2,
                                 name="gs")
                    for ch in range(NCH):
                        acc = mm_ps(f"g{l}{fe}{ch}")
                        for c in range(C):
                            nc.tensor.matmul(acc[:], wg[:, c, :],
                                             xbufs[ch][:, c, :], start=(c == 0),
                                             stop=(c == C - 1))
                        nc.scalar.activation(gs[:, SC * ch:SC * (ch + 1)], acc[:],
                                             AF.Silu)
                    wu = sb.tile([128, C, 128], dt.float16, tag="wp", bufs=2,
                                 name=f"wu{l}{fe}")
                    nc.sync.dma_start(wu[:], wguT[l, 1, fe])
                    for ch in range(NCH):
                        acc = mm_ps(f"u{l}{fe}{ch}")
                        for c in range(C):
                            nc.tensor.matmul(acc[:], wu[:, c, :],
                                             xbufs[ch][:, c, :], start=(c == 0),
                                             stop=(c == C - 1))
                        nc.vector.tensor_tensor(
                            actT[:, fe, SC * ch:SC * (ch + 1)], acc[:],
                            gs[:, SC * ch:SC * (ch + 1)], ALU.mult)
                # down proj
                for ch in range(NCH):
                  for quarter in range(4):
                    wd = sb.tile([128, FL // 128, 1024], dt.float16, tag="wp",
                                 bufs=2, name=f"wd{l}{ch}{quarter}")
                    nc.sync.dma_start(wd[:], wdT[l, quarter])
                    for do in range(8):
                        if True:
                            acc = mm_ps(f"wd{l}{quarter}{do}{ch}")
                            for fc in range(FL // 128):
                                nc.tensor.matmul(
                                    acc[:], wd[:, fc, 128 * do:128 * (do + 1)],
                                    actT[:, fc, SC * ch:SC * (ch + 1)],
                                    start=(fc == 0), stop=(fc == FL // 128 - 1))
                            ev = evh("wdev")
                            nc.vector.tensor_copy(ev[:], acc[:])
                            nc.sync.dma_start(
                                arin[ar_site][ch]
                                .rearrange("(t p) s -> p t s", p=128)
                                [:, 8 * quarter + do, :], ev[:])
                  nc.gpsimd.collective_compute(
                      "AllReduce", ALU.add,
                      replica_groups=[list(range(NCORES))],
                      ins=[arin[ar_site][ch][:]], outs=[arout[ar_site][ch][:]])

            # =========================================================
            # main flow
            # =========================================================
            def xb_tiles(nm):
                return [sb.tile([128, C, SC], dt.float16, tag="xb", bufs=2,
                                name=f"{nm}{ch}") for ch in range(NCH)]

            # layer 0
            x0 = xb_tiles("x0")
            rms_pass(h0T[:], None, None, 0, x0, "r0")
            attn_phase(0, x0, 0)
            x1 = xb_tiles("x1")
            rms_pass(h0T[:], arout[0], hdr[0][:], 1, x1, "r1")
            ffn_phase(0, x1, 1)
            # layer 1
            x2 = xb_tiles("x2")
            rms_pass(hdr[0][:], arout[1], hdr[1][:], 2, x2, "r2")
            attn_phase(1, x2, 2)
            x3 = xb_tiles("x3")
            rms_pass(hdr[1][:], arout[2], hdr[2][:], 3, x3, "r3")
            ffn_phase(1, x3, 3)
            # final rms + lm head
            xf = xb_tiles("xf")
            rms_pass(hdr[2][:], arout[3], None, 4, xf, "rf")
            for vq in range(4):
                lm = sb.tile([128, C, 256], dt.float16, tag="wp", bufs=2,
                             name=f"lm{vq}")
                nc.sync.dma_start(lm[:], lmT[vq])
                for vv in range(2):
                    for ch in range(NCH):
                        acc = mm_ps(f"lm{vq}{vv}{ch}")
                        for c in range(C):
                            nc.tensor.matmul(acc[:], lm[:, c, 128 * vv:128 * (vv + 1)],
                                             xf[ch][:, c, :], start=(c == 0),
                                             stop=(c == C - 1))
                        ev = evf("lmev")
                        nc.vector.tensor_copy(ev[:], acc[:])
                        nc.sync.dma_start(
                            logitsT[:].rearrange("(t p) s -> p t s", p=128)
                            [:, 2 * vq + vv, SC * ch:SC * (ch + 1)], ev[:])

    nc.finalize()
    return nc


_NC_CACHE = {}


def _get_module():
    if "nc" not in _NC_CACHE:
        _NC_CACHE["nc"] = build_module()
    return _NC_CACHE["nc"]


def _rope_tables():
    inv_freq = 1.0 / (ROPE_BASE ** (np.arange(0, DH, 2, dtype=np.float64) / DH))
    ang = np.arange(T, dtype=np.float64)[:, None] * inv_freq[None, :]
    emb = np.concatenate([ang, ang], axis=-1)          # [T, DH]
    return np.cos(emb).astype(np.float32), np.sin(emb).astype(np.float32)


def kernel(input_ids, memory, embed, Wq, Wk, Wv, Wo, Wg, Wu, Wd, Wmk, Wmv,
           ln1, ln2, normw, lm_head):
    input_ids = np.asarray(input_ids)
    f32 = np.float32
    memory = np.asarray(memory, f32)

    nc = _get_module()

    # host prep: embedding gather (pure data movement) + layout transforms
    h0 = np.asarray(embed, f32)[input_ids.reshape(-1)]          # [S, D]
    h0T = np.ascontiguousarray(h0.T)                            # [D, S] f32

    cos, sin = _rope_tables()
    qcs = np.stack([cos[M:], sin[M:]]).transpose(2, 0, 1)       # [128, 2, S]
    kcs = np.stack([cos, sin]).transpose(2, 0, 1)               # [128, 2, T]

    rmat = np.zeros((128, 128), f32)
    for d in range(64):
        rmat[d + 64, d] = -1.0
        rmat[d, d + 64] = 1.0

    tmaskv = np.full((128, 896), NEG, f32)
    for t in range(128):
        tmaskv[t, 384 + t:] = 0.0

    def bf(x):
        return np.ascontiguousarray(x).astype(BF)

    def swz(wT, nsplit):
        """[Din, n] (Din = c*128) -> [nsplit, 128, c, n/nsplit]."""
        c = wT.shape[0] // 128
        n = wT.shape[1]
        w = wT.reshape(c, 128, n).transpose(1, 0, 2)            # [128, c, n]
        w = w.reshape(128, c, nsplit, n // nsplit).transpose(2, 0, 1, 3)
        return w

    memT = np.stack([swz(memory[l, 0].T, 1)[0] for l in range(L)])

    in_maps = []
    for i in range(NCORES):
        hs = slice(DL * i, DL * (i + 1))
        fs = slice(FL * i, FL * (i + 1))
        vs = slice(VL * i, VL * (i + 1))
        lnw = np.stack([np.asarray(ln1, f32)[0], np.asarray(ln2, f32)[0],
                        np.asarray(ln1, f32)[1], np.asarray(ln2, f32)[1],
                        np.asarray(normw, f32)])                # [5, D]
        in_maps.append({
            "h0T": h0T,
            "memT": bf(memT),
            "wqkvT": bf(np.stack([np.stack([swz(np.asarray(W, f32)[l][hs].T, 2)
                                            for W in (Wq, Wk, Wv)])
                                  for l in range(L)])),
            "wmT": bf(np.stack([np.stack([swz(np.asarray(W, f32)[l][hs].T, 2)
                                          for W in (Wmk, Wmv)])
                                for l in range(L)])),
            "woT": bf(np.stack([swz(np.asarray(Wo, f32)[l][:, hs].T, 2)
                                for l in range(L)])),
            "wguT": bf(np.stack([np.stack([swz(np.asarray(W, f32)[l][fs].T, 8)
                                           for W in (Wg, Wu)])
                                 for l in range(L)])),
            "wdT": bf(np.stack([swz(np.asarray(Wd, f32)[l][:, fs].T, 4)
                                for l in range(L)])),
            "lmT": bf(swz(np.asarray(lm_head, f32)[vs].T, 4)),
            "qcs": bf(qcs),
            "kcs": bf(kcs),
            "rmat": bf(rmat),
            "tmask": bf(tmaskv),
            "lnw": np.ascontiguousarray(
                lnw.reshape(5, C, 128).transpose(2, 0, 1)),     # [128, 5, C]
        })

    res = run_bass_kernel_spmd(nc, in_maps, core_ids=list(range(NCORES)))
    _NC_CACHE["last_results"] = res

    logits = np.empty((B, S, V), f32)
    for i in range(NCORES):
        logits[0, :, VL * i:VL * (i + 1)] = res.results[i]["logitsT"].T
    return logits


# revision 23
# speedup vs baseline: 1.2522x; 1.1100x over previous
"""Trainium2 Bass kernel for a 2-layer LLaMA-style decoder with per-layer
memory K/V prefix (tokenmix2 Decoder), tensor-parallel over 8 NeuronCores.

Sharding: heads (32 -> 4/core), FFN intermediate (8192 -> 1024/core),
vocab (8192 -> 1024/core).  Two AllReduces per layer (attention out,
FFN out), each split into two sequence-chunk collectives for overlap.

Layouts: activations are kept transposed (xT [D, S]) so every matmul
output feeds the next without transposes.  Attention computes
scoresT [t, s] per head; softmax runs without max-subtraction (scores
are ~N(0,1) after the 1/sqrt(Dh) scale) with the normalization applied
on the oT psum evacuation.  Matmul compute in bf16, residual stream and
psum accumulation in fp32.
"""
import sys

sys.path.insert(0, "/opt/trn_rl_repo")

import numpy as np
import ml_dtypes

import concourse.bass as bass
import concourse.mybir as mybir
import concourse.tile as tile
from concourse import bacc
from concourse.bass_utils import run_bass_kernel_spmd

BF = np.float16

# model dims
L, D, H, DH, F, V = 2, 4096, 32, 128, 8192, 8192
B, S, M = 1, 1024, 512
T = M + S                      # 1536 total key positions
EPS = 1e-5
ROPE_BASE = 10000.0
SCALE = float(DH) ** -0.5

# per-core shards
NCORES = 8
HL = H // NCORES               # 4 local heads
DL = HL * DH                   # 512 local head dims
FL = F // NCORES               # 1024 local ffn
VL = V // NCORES               # 1024 local vocab
C = D // 128                   # 32 contraction tiles
NTT = T // 128                 # 12 key tiles
NMT = M // 128                 # 4 memory key tiles
NST = S // 128                 # 8 query tiles
NCH = 2                        # sequence chunks (512 each)
SC = S // NCH                  # 512

dt = mybir.dt
AF = mybir.ActivationFunctionType
ALU = mybir.AluOpType

NEG = -60000.0


def build_module():
    nc = bacc.Bacc("TRN2", target_bir_lowering=False, debug=False,
                   num_devices=NCORES)

    # const APs for activation scale/bias floats
    for v in (EPS, SCALE, 1.0 / D):
        t = nc.alloc_sbuf_tensor(f"cst_{v}", [128, 1], dt.float32)
        nc.gpsimd.memset(t.ap(), v)
        nc.const_aps.aps[(dt.float32, v)] = t.ap()

    # ---- kernel I/O ----
    h0T = nc.dram_tensor("h0T", [D, S], dt.float32, kind="ExternalInput")
    memT = nc.dram_tensor("memT", [L, 128, C, M], dt.float16, kind="ExternalInput")
    wqkvT = nc.dram_tensor("wqkvT", [L, 3, 2, 128, C, 256], dt.float16, kind="ExternalInput")
    wmT = nc.dram_tensor("wmT", [L, 2, 2, 128, C, 256], dt.float16, kind="ExternalInput")
    woT = nc.dram_tensor("woT", [L, 2, 128, HL, 2048], dt.float16, kind="ExternalInput")
    wguT = nc.dram_tensor("wguT", [L, 2, 8, 128, C, 128], dt.float16, kind="ExternalInput")
    wdT = nc.dram_tensor("wdT", [L, 4, 128, 8, 1024], dt.float16, kind="ExternalInput")
    lmT = nc.dram_tensor("lmT", [4, 128, C, 256], dt.float16, kind="ExternalInput")
    qcs = nc.dram_tensor("qcs", [128, 2, S], dt.float16, kind="ExternalInput")
    kcs = nc.dram_tensor("kcs", [128, 2, T], dt.float16, kind="ExternalInput")
    rmat_i = nc.dram_tensor("rmat", [128, 128], dt.float16, kind="ExternalInput")
    tmask = nc.dram_tensor("tmask", [128, 896], dt.float16, kind="ExternalInput")
    lnw = nc.dram_tensor("lnw", [128, 5, C], dt.float32, kind="ExternalInput")
    logitsT = nc.dram_tensor("logitsT", [VL, S], dt.float32, kind="ExternalOutput")

    with tile.TileContext(nc) as tc:
        with tc.tile_pool(name="sb", bufs=1) as sb, \
             tc.tile_pool(name="ps", bufs=1, space="PSUM") as ps, \
             tc.tile_pool(name="dr", bufs=1, space="DRAM") as dr:

            # ---- internal DRAM ----
            hdr = [dr.tile([D, S], dt.float32, tag=f"h{i}", bufs=1, name=f"h{i}")
                   for i in range(3)]           # h after resid 1..3
            arin = [[dr.tile([D, SC], dt.float16, tag=f"ai{i}{ch}", bufs=1,
                             name=f"ai{i}{ch}") for ch in range(NCH)]
                    for i in range(2 * L)]
            arout = [[dr.tile([D, SC], dt.float16, tag=f"ao{i}{ch}", bufs=1,
                              addr_space="Shared", name=f"ao{i}{ch}")
                      for ch in range(NCH)] for i in range(2 * L)]
            mkTd = [dr.tile([128, HL, M], dt.float16, tag=f"mk{l}", bufs=1,
                            name=f"mk{l}") for l in range(L)]
            mvd = [dr.tile([128, HL, NMT, DH], dt.float16, tag=f"mv{l}", bufs=1,
                           name=f"mv{l}") for l in range(L)]

            # ---- global constants in SBUF ----
            qc = sb.tile([128, 2, S], dt.float16, tag="qc", bufs=1, name="qc")
            nc.sync.dma_start(qc[:], qcs[:])
            kc = sb.tile([128, 2, T], dt.float16, tag="kc", bufs=1, name="kc")
            nc.sync.dma_start(kc[:], kcs[:])
            rmat = sb.tile([128, 128], dt.float16, tag="rm", bufs=1, name="rmat")
            nc.sync.dma_start(rmat[:], rmat_i[:])
            mask = sb.tile([128, 896], dt.float16, tag="msk", bufs=1, name="mask")
            nc.sync.dma_start(mask[:], tmask[:])
            lns = sb.tile([128, 5, C], dt.float32, tag="ln", bufs=1, name="lns")
            nc.sync.dma_start(lns[:], lnw[:])
            ones_bf = sb.tile([128, 1], dt.float16, tag="o1", bufs=1, name="ones_bf")
            nc.vector.memset(ones_bf[:], 1.0)
            ones_row = sb.tile([1, 128], dt.float16, tag="o2", bufs=1, name="ones_row")
            nc.vector.memset(ones_row[:], 1.0)
            inv64_row = sb.tile([1, 128], dt.float16, tag="o3", bufs=1, name="inv64_row")
            nc.vector.memset(inv64_row[:], 1.0 / 64.0)

            def mm_ps(name):
                return ps.tile([128, 512], dt.float32, tag="mm", bufs=4, name=name)

            def aux_ps(name):
                return ps.tile([1, 512], dt.float32, tag="aux", bufs=2, name=name)

            def evf(name):
                return sb.tile([128, 512], dt.float32, tag="evf", bufs=2, name=name)

            def evh(name):
                return sb.tile([128, 512], dt.float16, tag="evh", bufs=2, name=name)

            def rope_apply(raw_ps, cos_ap, sin_ap, out_ap):
                """raw_ps: [128,512] psum f32 (pre-rope head tile, d on part).
                Writes rope'd bf16 into out_ap."""
                raw_bf = sb.tile([128, 512], dt.float16, tag="rraw", bufs=2,
                                 name="raw_bf")
                nc.vector.tensor_copy(raw_bf[:], raw_ps[:])
                r_ps = mm_ps("r_ps")
                nc.tensor.matmul(r_ps[:], rmat[:], raw_bf[:], start=True, stop=True)
                m1 = sb.tile([128, 512], dt.float16, tag="rt", bufs=2, name="m1")
                nc.vector.tensor_tensor(m1[:], raw_bf[:], cos_ap, ALU.mult)
                m2 = sb.tile([128, 512], dt.float16, tag="rt2", bufs=2, name="m2")
                nc.vector.tensor_tensor(m2[:], r_ps[:], sin_ap, ALU.mult)
                nc.vector.tensor_tensor(out_ap, m1[:], m2[:], ALU.add)

            # =========================================================
            # pre-phase: memory K/V projections for both layers -> DRAM
            # =========================================================
            for l in range(L):
                mem_sb = sb.tile([128, C, M], dt.float16, tag="xb", bufs=2,
                                 name=f"mem{l}")
                nc.sync.dma_start(mem_sb[:], memT[l])
                # mk: for each local head tile d -> [128, 512] then rope
                for half in range(2):
                    wmk = sb.tile([128, C, 256], dt.float16, tag="wp", bufs=2,
                                  name=f"wmk{l}{half}")
                    nc.sync.dma_start(wmk[:], wmT[l, 0, half])
                    for dd in range(2):
                        d = 2 * half + dd
                        acc = mm_ps(f"mk{l}{d}")
                        for c in range(C):
                            nc.tensor.matmul(acc[:], wmk[:, c, 128 * dd:128 * (dd + 1)],
                                             mem_sb[:, c, :], start=(c == 0),
                                             stop=(c == C - 1))
                        mko = sb.tile([128, 512], dt.float16, tag="pt", bufs=3,
                                      name="mko")
                        rope_apply(acc, kc[:, 0, :M], kc[:, 1, :M], mko[:])
                        nc.sync.dma_start(mkTd[l][:, d, :], mko[:])
                # mv: natural layout [m, d]
                for half in range(2):
                    wmv = sb.tile([128, C, 256], dt.float16, tag="wp", bufs=2,
                                  name=f"wmv{l}{half}")
                    nc.sync.dma_start(wmv[:], wmT[l, 1, half])
                    for mt in range(NMT):
                        acc = mm_ps(f"mv{l}{half}{mt}")
                        for c in range(C):
                            nc.tensor.matmul(acc[:, :256],
                                             mem_sb[:, c, 128 * mt:128 * (mt + 1)],
                                             wmv[:, c, :], start=(c == 0),
                                             stop=(c == C - 1))
                        mvo = sb.tile([128, 2, 128], dt.float16, tag="mvo", bufs=3,
                                      name="mvo")
                        nc.vector.tensor_copy(
                            mvo[:], acc[:, :256].rearrange("p (h d) -> p h d", d=128))
                        nc.sync.dma_start(mvd[l][:, 2 * half:2 * half + 2, mt, :],
                                          mvo[:])

            # =========================================================
            # rms pass: h_new = h_src (+ delta); write h_dst; xT = rms
            # =========================================================
            def rms_pass(h_src, delta, h_dst, ln_idx, xbufs, name):
                """h_src: DRAM [D, S] f32 AP; delta: list per chunk of DRAM
                [D, SC] or None; h_dst same form or None; xbufs: list per chunk
                of SBUF tiles [128, C, SC] bf16 (written in place)."""
                hv = h_src.rearrange("(c p) s -> p c s", p=128)
                for ch in range(NCH):
                    xb = xbufs[ch]
                    ssq = aux_ps(f"ssq_{name}{ch}")
                    for c in range(C):
                        ht = sb.tile([128, 512], dt.float32, tag="hl", bufs=2,
                                     name="ht")
                        nc.sync.dma_start(ht[:], hv[:, c, SC * ch:SC * (ch + 1)])
                        if delta is not None:
                            dtl = sb.tile([128, 512], dt.float16, tag="dl", bufs=2,
                                          name="dtl")
                            nc.sync.dma_start(
                                dtl[:],
                                delta[ch].rearrange("(c p) s -> p c s", p=128)[:, c, :])
                            hn = sb.tile([128, 512], dt.float32, tag="hn", bufs=2,
                                         name="hn")
                            nc.vector.tensor_tensor(hn[:], ht[:], dtl[:], ALU.add)
                            if h_dst is not None:
                                nc.sync.dma_start(
                                    h_dst.rearrange("(c p) s -> p c s", p=128)
                                    [:, c, SC * ch:SC * (ch + 1)], hn[:])
                        else:
                            hn = ht
                        # fp16 copy for pass 2 (in xb) on ACT, square on DVE
                        nc.scalar.copy(xb[:, c, :], hn[:])
                        hsq = sb.tile([128, 512], dt.float16, tag="hsq", bufs=2,
                                      name="hsq")
                        nc.vector.tensor_tensor(hsq[:], hn[:], hn[:], ALU.mult)
                        nc.tensor.matmul(ssq[:], ones_bf[:], hsq[:],
                                         start=(c == 0), stop=(c == C - 1))
                    # rsqrt row and broadcast
                    sq = sb.tile([1, 512], dt.float32, tag="row", bufs=2, name="sq")
                    nc.scalar.activation(sq[:], ssq[:], AF.Sqrt, bias=EPS,
                                         scale=1.0 / D)
                    rs = sb.tile([1, 512], dt.float16, tag="row2", bufs=2, name="rs")
                    with nc.allow_low_precision(reason="fp16 row for broadcast mm"):
                        nc.vector.reciprocal(rs[:], sq[:])
                    bc = ps.tile([128, 512], dt.float32, tag="bc", bufs=2, name="bc")
                    nc.tensor.matmul(bc[:], ones_row[:], rs[:], start=True,
                                     stop=True)
                    for c in range(C):
                        nc.vector.scalar_tensor_tensor(
                            xb[:, c, :], xb[:, c, :], lns[:, ln_idx, c:c + 1],
                            bc[:], ALU.mult, ALU.mult)

            # =========================================================
            # attention + Wo for one layer; xbufs hold xT
            # =========================================================
            def attn_phase(l, xbufs, ar_site):
                # KT per head / V built first (k, v, then per-head q + attn)
                KT = sb.tile([128, HL, T], dt.float16, tag="KT", bufs=1,
                             name=f"KT{l}")
                Vt = sb.tile([128, HL, NTT, DH], dt.float16, tag="V", bufs=1,
                             name=f"V{l}")
                nc.sync.dma_start(KT[:, :, :M], mkTd[l][:])
                nc.sync.dma_start(Vt[:, :, :NMT, :], mvd[l][:])
                # k/v projections, chunk-major so chunk 0 streams while
                # chunk 1's AR + rms still run
                for ch in range(NCH):
                    for half in range(2):
                        wk = sb.tile([128, C, 256], dt.float16, tag="wp", bufs=2,
                                     name=f"wk{l}{ch}{half}")
                        nc.sync.dma_start(wk[:], wqkvT[l, 1, half])
                        for dd in range(2):
                            d = 2 * half + dd
                            acc = mm_ps(f"k{l}{d}{ch}")
                            for c in range(C):
                                nc.tensor.matmul(
                                    acc[:], wk[:, c, 128 * dd:128 * (dd + 1)],
                                    xbufs[ch][:, c, :], start=(c == 0),
                                    stop=(c == C - 1))
                            rope_apply(acc, kc[:, 0, M + SC * ch:M + SC * (ch + 1)],
                                       kc[:, 1, M + SC * ch:M + SC * (ch + 1)],
                                       KT[:, d, M + SC * ch:M + SC * (ch + 1)])
                    for half in range(2):
                        wv = sb.tile([128, C, 256], dt.float16, tag="wp", bufs=2,
                                     name=f"wv{l}{ch}{half}")
                        nc.sync.dma_start(wv[:], wqkvT[l, 2, half])
                        for sti in range(4):
                            st = 4 * ch + sti
                            acc = mm_ps(f"v{l}{half}{st}")
                            for c in range(C):
                                nc.tensor.matmul(
                                    acc[:, :256],
                                    xbufs[ch][:, c, 128 * sti:128 * (sti + 1)],
                                    wv[:, c, :], start=(c == 0), stop=(c == C - 1))
                            nc.vector.tensor_copy(
                                Vt[:, 2 * half:2 * half + 2, NMT + st, :],
                                acc[:, :256].rearrange("p (h d) -> p h d", d=128))
                # per-head: q proj + attention
                oT = sb.tile([128, HL, S], dt.float16, tag="oT", bufs=1,
                             name=f"oT{l}")
                for half in range(2):
                    wqh = sb.tile([128, C, 256], dt.float16, tag="wp", bufs=2,
                                  name=f"wq{l}{half}")
                    nc.sync.dma_start(wqh[:], wqkvT[l, 0, half])
                    for hh in range(2):
                        h = 2 * half + hh
                        qT = sb.tile([128, S], dt.float16, tag="qT", bufs=2,
                                     name=f"qT{l}{h}")
                        for ch in range(NCH):
                            acc = mm_ps(f"q{l}{h}{ch}")
                            for c in range(C):
                                nc.tensor.matmul(
                                    acc[:], wqh[:, c, 128 * hh:128 * (hh + 1)],
                                    xbufs[ch][:, c, :], start=(c == 0),
                                    stop=(c == C - 1))
                            rope_apply(acc, qc[:, 0, SC * ch:SC * (ch + 1)],
                                       qc[:, 1, SC * ch:SC * (ch + 1)],
                                       qT[:, SC * ch:SC * (ch + 1)])
                        for sb_i in range(NCH):
                            ntt = NMT + 4 * (sb_i + 1)
                            o_ps = mm_ps(f"o{l}{h}{sb_i}")
                            s_ps = aux_ps(f"s{l}{h}{sb_i}")
                            for tt in range(ntt):
                                sc_ps = mm_ps(f"sc{l}{h}{sb_i}{tt}")
                                nc.tensor.matmul(sc_ps[:],
                                                 KT[:, h, 128 * tt:128 * (tt + 1)],
                                                 qT[:, SC * sb_i:SC * (sb_i + 1)],
                                                 start=True, stop=True)
                                dtile = tt - ntt + 4      # >= 0 -> diagonal tile
                                if dtile >= 0:
                                    off = 384 - 128 * dtile
                                    nc.vector.tensor_tensor(
                                        sc_ps[:], sc_ps[:],
                                        mask[:, off:off + 512], ALU.add)
                                pt = sb.tile([128, 512], dt.float16, tag="pt",
                                             bufs=3, name="pt")
                                nc.scalar.activation(pt[:], sc_ps[:], AF.Exp,
                                                     scale=SCALE)
                                nc.tensor.matmul(o_ps[:], Vt[:, h, tt, :], pt[:],
                                                 start=(tt == 0),
                                                 stop=(tt == ntt - 1))
                                nc.tensor.matmul(s_ps[:], ones_bf[:], pt[:],
                                                 start=(tt == 0),
                                                 stop=(tt == ntt - 1))
                            rrf = sb.tile([1, 512], dt.float32, tag="rowf", bufs=2,
                                          name="rrf")
                            nc.vector.reciprocal(rrf[:], s_ps[:])
                            rr = sb.tile([1, 512], dt.float16, tag="row2", bufs=2,
                                         name="rr")
                            with nc.allow_low_precision(reason="fp16 row for broadcast mm"):
                                nc.vector.tensor_scalar_mul(rr[:], rrf[:], 64.0)
                            bc = ps.tile([128, 512], dt.float32, tag="bc", bufs=2,
                                         name="bca")
                            nc.tensor.matmul(bc[:], inv64_row[:], rr[:],
                                             start=True, stop=True)
                            bcs = sb.tile([128, 512], dt.float32, tag="bcs",
                                          bufs=2, name="bcs")
                            nc.vector.tensor_copy(bcs[:], bc[:])
                            nc.vector.tensor_tensor(
                                oT[:, h, SC * sb_i:SC * (sb_i + 1)],
                                o_ps[:], bcs[:], ALU.mult)
                # Wo: out [Do, s] partial sums -> arin
                for ch in range(NCH):
                  for half in range(2):
                    wo = sb.tile([128, HL, 2048], dt.float16, tag="wp", bufs=2,
                                 name=f"wo{l}{ch}{half}")
                    nc.sync.dma_start(wo[:], woT[l, half])
                    for do in range(16):
                        if True:
                            acc = mm_ps(f"wo{l}{half}{do}{ch}")
                            for hh in range(HL):
                                nc.tensor.matmul(
                                    acc[:], wo[:, hh, 128 * do:128 * (do + 1)],
                                    oT[:, hh, SC * ch:SC * (ch + 1)],
                                    start=(hh == 0), stop=(hh == HL - 1))
                            ev = evh("woev")
                            nc.vector.tensor_copy(ev[:], acc[:])
                            nc.sync.dma_start(
                                arin[ar_site][ch]
                                .rearrange("(t p) s -> p t s", p=128)
                                [:, 16 * half + do, :], ev[:])
                  nc.gpsimd.collective_compute(
                      "AllReduce", ALU.add,
                      replica_groups=[list(range(NCORES))],
                      ins=[arin[ar_site][ch][:]], outs=[arout[ar_site][ch][:]])

            # =========================================================
            # FFN for one layer: xbufs -> partial down-proj -> arin
            # =========================================================
            def ffn_phase(l, xbufs, ar_site):
                actT = sb.tile([128, FL // 128, S], dt.float16, tag="actT",
                               bufs=1, name=f"actT{l}")
                for ch in range(NCH):
                    for fe in range(FL // 128):
                        wg = sb.tile([128, C, 128], dt.float16, tag="wp", bufs=2,
                                     name=f"wg{l}{ch}{fe}")
                        nc.sync.dma_start(wg[:], wguT[l, 0, fe])
                        gs = sb.tile([128, 512], dt.float16, tag="gs", bufs=2,
                                     name="gs")
                        acc = mm_ps(f"g{l}{fe}{ch}")
                        for c in range(C):
                            nc.tensor.matmul(acc[:], wg[:, c, :],
                                             xbufs[ch][:, c, :], start=(c == 0),
                                             stop=(c == C - 1))
                        nc.scalar.activation(gs[:], acc[:], AF.Silu)
                        wu = sb.tile([128, C, 128], dt.float16, tag="wp", bufs=2,
                                     name=f"wu{l}{ch}{fe}")
                        nc.sync.dma_start(wu[:], wguT[l, 1, fe])
                        acc2 = mm_ps(f"u{l}{fe}{ch}")
                        for c in range(C):
                            nc.tensor.matmul(acc2[:], wu[:, c, :],
                                             xbufs[ch][:, c, :], start=(c == 0),
                                             stop=(c == C - 1))
                        nc.vector.tensor_tensor(
                            actT[:, fe, SC * ch:SC * (ch + 1)], acc2[:],
                            gs[:], ALU.mult)
                # down proj
                for ch in range(NCH):
                  for quarter in range(4):
                    wd = sb.tile([128, FL // 128, 1024], dt.float16, tag="wp",
                                 bufs=2, name=f"wd{l}{ch}{quarter}")
                    nc.sync.dma_start(wd[:], wdT[l, quarter])
                    for do in range(8):
                        if True:
                            acc = mm_ps(f"wd{l}{quarter}{do}{ch}")
                            for fc in range(FL // 128):
                                nc.tensor.matmul(
                                    acc[:], wd[:, fc, 128 * do:128 * (do + 1)],
                                    actT[:, fc, SC * ch:SC * (ch + 1)],
                                    start=(fc == 0), stop=(fc == FL // 128 - 1))
                            ev = evh("wdev")
                            nc.vector.tensor_copy(ev[:], acc[:])
                            nc.sync.dma_start(
                                arin[ar_site][ch]
                                .rearrange("(t p) s -> p t s", p=128)
                                [:, 8 * quarter + do, :], ev[:])
                  nc.gpsimd.collective_compute(
                      "AllReduce", ALU.add,
                      replica_groups=[list(range(NCORES))],
                      ins=[arin[ar_site][ch][:]], outs=[arout[ar_site][ch][:]])

            # =========================================================
            # main flow
            # =========================================================
            def xb_tiles(nm):
                return [sb.tile([128, C, SC], dt.float16, tag="xb", bufs=2,
                                name=f"{nm}{ch}") for ch in range(NCH)]

            # layer 0
            x0 = xb_tiles("x0")
            rms_pass(h0T[:], None, None, 0, x0, "r0")
            attn_phase(0, x0, 0)
            x1 = xb_tiles("x1")
            rms_pass(h0T[:], arout[0], hdr[0][:], 1, x1, "r1")
            ffn_phase(0, x1, 1)
            # layer 1
            x2 = xb_tiles("x2")
            rms_pass(hdr[0][:], arout[1], hdr[1][:], 2, x2, "r2")
            attn_phase(1, x2, 2)
            x3 = xb_tiles("x3")
            rms_pass(hdr[1][:], arout[2], hdr[2][:], 3, x3, "r3")
            ffn_phase(1, x3, 3)
            # final rms + lm head
            xf = xb_tiles("xf")
            rms_pass(hdr[2][:], arout[3], None, 4, xf, "rf")
            for ch in range(NCH):
              for vq in range(4):
                lm = sb.tile([128, C, 256], dt.float16, tag="wp", bufs=2,
                             name=f"lm{ch}{vq}")
                nc.sync.dma_start(lm[:], lmT[vq])
                for vv in range(2):
                    if True:
                        acc = mm_ps(f"lm{vq}{vv}{ch}")
                        for c in range(C):
                            nc.tensor.matmul(acc[:], lm[:, c, 128 * vv:128 * (vv + 1)],
                                             xf[ch][:, c, :], start=(c == 0),
                                             stop=(c == C - 1))
                        ev = evf("lmev")
                        nc.vector.tensor_copy(ev[:], acc[:])
                        nc.sync.dma_start(
                            logitsT[:].rearrange("(t p) s -> p t s", p=128)
                            [:, 2 * vq + vv, SC * ch:SC * (ch + 1)], ev[:])

    nc.finalize()
    return nc


_NC_CACHE = {}


def _get_module():
    if "nc" not in _NC_CACHE:
        _NC_CACHE["nc"] = build_module()
    return _NC_CACHE["nc"]


def _rope_tables():
    inv_freq = 1.0 / (ROPE_BASE ** (np.arange(0, DH, 2, dtype=np.float64) / DH))
    ang = np.arange(T, dtype=np.float64)[:, None] * inv_freq[None, :]
    emb = np.concatenate([ang, ang], axis=-1)          # [T, DH]
    return np.cos(emb).astype(np.float32), np.sin(emb).astype(np.float32)


def kernel(input_ids, memory, embed, Wq, Wk, Wv, Wo, Wg, Wu, Wd, Wmk, Wmv,
           ln1, ln2, normw, lm_head):
    input_ids = np.asarray(input_ids)
    f32 = np.float32
    memory = np.asarray(memory, f32)

    nc = _get_module()

    # host prep: embedding gather (pure data movement) + layout transforms
    h0 = np.asarray(embed, f32)[input_ids.reshape(-1)]          # [S, D]
    h0T = np.ascontiguousarray(h0.T)                            # [D, S] f32

    cos, sin = _rope_tables()
    qcs = np.stack([cos[M:], sin[M:]]).transpose(2, 0, 1)       # [128, 2, S]
    kcs = np.stack([cos, sin]).transpose(2, 0, 1)               # [128, 2, T]

    rmat = np.zeros((128, 128), f32)
    for d in range(64):
        rmat[d + 64, d] = -1.0
        rmat[d, d + 64] = 1.0

    tmaskv = np.full((128, 896), NEG, f32)
    for t in range(128):
        tmaskv[t, 384 + t:] = 0.0

    def bf(x):
        return np.ascontiguousarray(x).astype(BF)

    def swz(wT, nsplit):
        """[Din, n] (Din = c*128) -> [nsplit, 128, c, n/nsplit]."""
        c = wT.shape[0] // 128
        n = wT.shape[1]
        w = wT.reshape(c, 128, n).transpose(1, 0, 2)            # [128, c, n]
        w = w.reshape(128, c, nsplit, n // nsplit).transpose(2, 0, 1, 3)
        return w

    memT = np.stack([swz(memory[l, 0].T, 1)[0] for l in range(L)])

    in_maps = []
    for i in range(NCORES):
        hs = slice(DL * i, DL * (i + 1))
        fs = slice(FL * i, FL * (i + 1))
        vs = slice(VL * i, VL * (i + 1))
        lnw = np.stack([np.asarray(ln1, f32)[0], np.asarray(ln2, f32)[0],
                        np.asarray(ln1, f32)[1], np.asarray(ln2, f32)[1],
                        np.asarray(normw, f32)])                # [5, D]
        in_maps.append({
            "h0T": h0T,
            "memT": bf(memT),
            "wqkvT": bf(np.stack([np.stack([swz(np.asarray(W, f32)[l][hs].T, 2)
                                            for W in (Wq, Wk, Wv)])
                                  for l in range(L)])),
            "wmT": bf(np.stack([np.stack([swz(np.asarray(W, f32)[l][hs].T, 2)
                                          for W in (Wmk, Wmv)])
                                for l in range(L)])),
            "woT": bf(np.stack([swz(np.asarray(Wo, f32)[l][:, hs].T, 2)
                                for l in range(L)])),
            "wguT": bf(np.stack([np.stack([swz(np.asarray(W, f32)[l][fs].T, 8)
                                           for W in (Wg, Wu)])
                                 for l in range(L)])),
            "wdT": bf(np.stack([swz(np.asarray(Wd, f32)[l][:, fs].T, 4)
                                for l in range(L)])),
            "lmT": bf(swz(np.asarray(lm_head, f32)[vs].T, 4)),
            "qcs": bf(qcs),
            "kcs": bf(kcs),
            "rmat": bf(rmat),
            "tmask": bf(tmaskv),
            "lnw": np.ascontiguousarray(
                lnw.reshape(5, C, 128).transpose(2, 0, 1)),     # [128, 5, C]
        })

    res = run_bass_kernel_spmd(nc, in_maps, core_ids=list(range(NCORES)))
    _NC_CACHE["last_results"] = res

    logits = np.empty((B, S, V), f32)
    for i in range(NCORES):
        logits[0, :, VL * i:VL * (i + 1)] = res.results[i]["logitsT"].T
    return logits


# revision 26
# speedup vs baseline: 1.3804x; 1.1024x over previous
"""Trainium2 Bass kernel for a 2-layer LLaMA-style decoder with per-layer
memory K/V prefix (tokenmix2 Decoder), tensor-parallel over 8 NeuronCores.

Sharding: heads (32 -> 4/core), FFN intermediate (8192 -> 1024/core),
vocab (8192 -> 1024/core).  Two AllReduces per layer (attention out,
FFN out), each split into two sequence-chunk collectives for overlap.

Layouts: activations are kept transposed (xT [D, S]) so every matmul
output feeds the next without transposes.  Attention computes
scoresT [t, s] per head; softmax runs without max-subtraction (scores
are ~N(0,1) after the 1/sqrt(Dh) scale) with the normalization applied
on the oT psum evacuation.  Matmul compute in bf16, residual stream and
psum accumulation in fp32.
"""
import sys

sys.path.insert(0, "/opt/trn_rl_repo")

import numpy as np
import ml_dtypes

import concourse.bass as bass
import concourse.mybir as mybir
import concourse.tile as tile
from concourse import bacc
from concourse.bass_utils import run_bass_kernel_spmd

BF = np.float16

# model dims
L, D, H, DH, F, V = 2, 4096, 32, 128, 8192, 8192
B, S, M = 1, 1024, 512
T = M + S                      # 1536 total key positions
EPS = 1e-5
ROPE_BASE = 10000.0
SCALE = float(DH) ** -0.5

# per-core shards
NCORES = 8
HL = H // NCORES               # 4 local heads
DL = HL * DH                   # 512 local head dims
FL = F // NCORES               # 1024 local ffn
VL = V // NCORES               # 1024 local vocab
C = D // 128                   # 32 contraction tiles
NTT = T // 128                 # 12 key tiles
NMT = M // 128                 # 4 memory key tiles
NST = S // 128                 # 8 query tiles
NCH = 2                        # sequence chunks (512 each)
SC = S // NCH                  # 512

dt = mybir.dt
AF = mybir.ActivationFunctionType
ALU = mybir.AluOpType

NEG = -60000.0


def build_module():
    nc = bacc.Bacc("TRN2", target_bir_lowering=False, debug=False,
                   num_devices=NCORES)

    # const APs for activation scale/bias floats
    for v in (EPS, SCALE, 1.0 / D):
        t = nc.alloc_sbuf_tensor(f"cst_{v}", [128, 1], dt.float32)
        nc.gpsimd.memset(t.ap(), v)
        nc.const_aps.aps[(dt.float32, v)] = t.ap()

    # ---- kernel I/O ----
    h0T = nc.dram_tensor("h0T", [D, S], dt.float16, kind="ExternalInput")
    memT = nc.dram_tensor("memT", [L, 128, C, M], dt.float16, kind="ExternalInput")
    wqkvT = nc.dram_tensor("wqkvT", [L, 3, 2, 128, C, 256], dt.float16, kind="ExternalInput")
    wmT = nc.dram_tensor("wmT", [L, 2, 2, 128, C, 256], dt.float16, kind="ExternalInput")
    woT = nc.dram_tensor("woT", [L, 2, 128, HL, 2048], dt.float16, kind="ExternalInput")
    wguT = nc.dram_tensor("wguT", [L, 2, 8, 128, C, 128], dt.float16, kind="ExternalInput")
    wdT = nc.dram_tensor("wdT", [L, 4, 128, 8, 1024], dt.float16, kind="ExternalInput")
    lmT = nc.dram_tensor("lmT", [4, 128, C, 256], dt.float16, kind="ExternalInput")
    qcs = nc.dram_tensor("qcs", [128, 2, S], dt.float16, kind="ExternalInput")
    kcs = nc.dram_tensor("kcs", [128, 2, T], dt.float16, kind="ExternalInput")
    rmat_i = nc.dram_tensor("rmat", [128, 128], dt.float16, kind="ExternalInput")
    tmask = nc.dram_tensor("tmask", [128, 896], dt.float16, kind="ExternalInput")
    lnw = nc.dram_tensor("lnw", [128, 5, C], dt.float32, kind="ExternalInput")
    logitsT = nc.dram_tensor("logitsT", [VL, S], dt.float32, kind="ExternalOutput")

    with tile.TileContext(nc) as tc:
        with tc.tile_pool(name="sb", bufs=1) as sb, \
             tc.tile_pool(name="ps", bufs=1, space="PSUM") as ps, \
             tc.tile_pool(name="dr", bufs=1, space="DRAM") as dr:

            # ---- internal DRAM ----
            hdr = [dr.tile([D, S], dt.float16, tag=f"h{i}", bufs=1, name=f"h{i}")
                   for i in range(3)]           # h after resid 1..3
            arin = [[dr.tile([D, SC], dt.float16, tag=f"ai{i}{ch}", bufs=1,
                             name=f"ai{i}{ch}") for ch in range(NCH)]
                    for i in range(2 * L)]
            arout = [[dr.tile([D, SC], dt.float16, tag=f"ao{i}{ch}", bufs=1,
                              addr_space="Shared", name=f"ao{i}{ch}")
                      for ch in range(NCH)] for i in range(2 * L)]
            mkTd = [dr.tile([128, HL, M], dt.float16, tag=f"mk{l}", bufs=1,
                            name=f"mk{l}") for l in range(L)]
            mvd = [dr.tile([128, HL, NMT, DH], dt.float16, tag=f"mv{l}", bufs=1,
                           name=f"mv{l}") for l in range(L)]

            # ---- global constants in SBUF ----
            qc = sb.tile([128, 2, S], dt.float16, tag="qc", bufs=1, name="qc")
            nc.sync.dma_start(qc[:], qcs[:])
            kc = sb.tile([128, 2, T], dt.float16, tag="kc", bufs=1, name="kc")
            nc.sync.dma_start(kc[:], kcs[:])
            rmat = sb.tile([128, 128], dt.float16, tag="rm", bufs=1, name="rmat")
            nc.sync.dma_start(rmat[:], rmat_i[:])
            mask = sb.tile([128, 896], dt.float16, tag="msk", bufs=1, name="mask")
            nc.sync.dma_start(mask[:], tmask[:])
            lns = sb.tile([128, 5, C], dt.float32, tag="ln", bufs=1, name="lns")
            nc.sync.dma_start(lns[:], lnw[:])
            ones_bf = sb.tile([128, 1], dt.float16, tag="o1", bufs=1, name="ones_bf")
            nc.vector.memset(ones_bf[:], 1.0)
            ones_row = sb.tile([1, 128], dt.float16, tag="o2", bufs=1, name="ones_row")
            nc.vector.memset(ones_row[:], 1.0)
            inv64_row = sb.tile([1, 128], dt.float16, tag="o3", bufs=1, name="inv64_row")
            nc.vector.memset(inv64_row[:], 1.0 / 64.0)

            def mm_ps(name):
                return ps.tile([128, 512], dt.float32, tag="mm", bufs=5, name=name)

            def aux_ps(name):
                return ps.tile([1, 512], dt.float32, tag="aux", bufs=1, name=name)

            def evf(name):
                return sb.tile([128, 512], dt.float32, tag="evf", bufs=1, name=name)

            def evh(name):
                return sb.tile([128, 512], dt.float16, tag="evh", bufs=2, name=name)

            def rope_apply(raw_ps, cos_ap, sin_ap, out_ap):
                """raw_ps: [128,512] psum f32 (pre-rope head tile, d on part).
                Writes rope'd bf16 into out_ap."""
                raw_bf = sb.tile([128, 512], dt.float16, tag="rraw", bufs=2,
                                 name="raw_bf")
                nc.vector.tensor_copy(raw_bf[:], raw_ps[:])
                r_ps = mm_ps("r_ps")
                nc.tensor.matmul(r_ps[:], rmat[:], raw_bf[:], start=True, stop=True)
                m1 = sb.tile([128, 512], dt.float16, tag="rt", bufs=2, name="m1")
                nc.vector.tensor_tensor(m1[:], raw_bf[:], cos_ap, ALU.mult)
                m2 = sb.tile([128, 512], dt.float16, tag="rt2", bufs=2, name="m2")
                nc.vector.tensor_tensor(m2[:], r_ps[:], sin_ap, ALU.mult)
                nc.vector.tensor_tensor(out_ap, m1[:], m2[:], ALU.add)

            # =========================================================
            # pre-phase: memory K/V projections for both layers -> DRAM
            # =========================================================
            for l in range(L):
                mem_sb = sb.tile([128, C, M], dt.float16, tag="xb", bufs=2,
                                 name=f"mem{l}")
                nc.sync.dma_start(mem_sb[:], memT[l])
                # mk: for each local head tile d -> [128, 512] then rope
                for half in range(2):
                    wmk = sb.tile([128, C, 256], dt.float16, tag="wp", bufs=2,
                                  name=f"wmk{l}{half}")
                    nc.sync.dma_start(wmk[:], wmT[l, 0, half])
                    for dd in range(2):
                        d = 2 * half + dd
                        acc = mm_ps(f"mk{l}{d}")
                        for c in range(C):
                            nc.tensor.matmul(acc[:], wmk[:, c, 128 * dd:128 * (dd + 1)],
                                             mem_sb[:, c, :], start=(c == 0),
                                             stop=(c == C - 1))
                        mko = sb.tile([128, 512], dt.float16, tag="pt", bufs=3,
                                      name="mko")
                        rope_apply(acc, kc[:, 0, :M], kc[:, 1, :M], mko[:])
                        nc.sync.dma_start(mkTd[l][:, d, :], mko[:])
                # mv: natural layout [m, d]
                for half in range(2):
                    wmv = sb.tile([128, C, 256], dt.float16, tag="wp", bufs=2,
                                  name=f"wmv{l}{half}")
                    nc.sync.dma_start(wmv[:], wmT[l, 1, half])
                    for mt in range(NMT):
                        acc = mm_ps(f"mv{l}{half}{mt}")
                        for c in range(C):
                            nc.tensor.matmul(acc[:, :256],
                                             mem_sb[:, c, 128 * mt:128 * (mt + 1)],
                                             wmv[:, c, :], start=(c == 0),
                                             stop=(c == C - 1))
                        mvo = sb.tile([128, 2, 128], dt.float16, tag="mvo", bufs=3,
                                      name="mvo")
                        nc.vector.tensor_copy(
                            mvo[:], acc[:, :256].rearrange("p (h d) -> p h d", d=128))
                        nc.sync.dma_start(mvd[l][:, 2 * half:2 * half + 2, mt, :],
                                          mvo[:])

            # =========================================================
            # rms pass: h_new = h_src (+ delta); write h_dst; xT = rms
            # =========================================================
            def rms_pass(h_src, delta, h_dst, ln_idx, xbufs, name):
                """h_src: DRAM [D, S] f32 AP; delta: list per chunk of DRAM
                [D, SC] or None; h_dst same form or None; xbufs: list per chunk
                of SBUF tiles [128, C, SC] bf16 (written in place)."""
                hv = h_src.rearrange("(c p) s -> p c s", p=128)
                for ch in range(NCH):
                    xb = xbufs[ch]
                    ssq = aux_ps(f"ssq_{name}{ch}")
                    for cq in range(C // 4):
                        csl = slice(4 * cq, 4 * cq + 4)
                        if delta is None:
                            # h goes straight into xb (fp16 -> fp16 DMA)
                            nc.sync.dma_start(
                                xb[:, csl, :],
                                hv[:, csl, SC * ch:SC * (ch + 1)])
                        else:
                            ht = sb.tile([128, 4, 512], dt.float16, tag="hl",
                                         bufs=2, name="ht")
                            nc.sync.dma_start(ht[:],
                                              hv[:, csl, SC * ch:SC * (ch + 1)])
                            dtl = sb.tile([128, 4, 512], dt.float16, tag="dl",
                                          bufs=2, name="dtl")
                            nc.sync.dma_start(
                                dtl[:],
                                delta[ch].rearrange("(c p) s -> p c s", p=128)
                                [:, csl, :])
                            for ci in range(4):
                                nc.vector.tensor_tensor(xb[:, 4 * cq + ci, :],
                                                        ht[:, ci, :],
                                                        dtl[:, ci, :], ALU.add)
                            if h_dst is not None:
                                nc.sync.dma_start(
                                    h_dst.rearrange("(c p) s -> p c s", p=128)
                                    [:, csl, SC * ch:SC * (ch + 1)],
                                    xb[:, csl, :])
                        for ci in range(4):
                            c = 4 * cq + ci
                            hsq = sb.tile([128, 512], dt.float16, tag="hsq",
                                          bufs=2, name="hsq")
                            nc.vector.tensor_tensor(hsq[:], xb[:, c, :],
                                                    xb[:, c, :], ALU.mult)
                            nc.tensor.matmul(ssq[:], ones_bf[:], hsq[:],
                                             start=(c == 0), stop=(c == C - 1))
                    # rsqrt row and broadcast
                    sq = sb.tile([1, 512], dt.float32, tag="row", bufs=2, name="sq")
                    nc.scalar.activation(sq[:], ssq[:], AF.Sqrt, bias=EPS,
                                         scale=1.0 / D)
                    rs = sb.tile([1, 512], dt.float16, tag="row2", bufs=2, name="rs")
                    with nc.allow_low_precision(reason="fp16 row for broadcast mm"):
                        nc.vector.reciprocal(rs[:], sq[:])
                    bc = ps.tile([128, 512], dt.float32, tag="bc", bufs=2, name="bc")
                    nc.tensor.matmul(bc[:], ones_row[:], rs[:], start=True,
                                     stop=True)
                    for c in range(C):
                        nc.vector.scalar_tensor_tensor(
                            xb[:, c, :], xb[:, c, :], lns[:, ln_idx, c:c + 1],
                            bc[:], ALU.mult, ALU.mult)

            # =========================================================
            # attention + Wo for one layer; xbufs hold xT
            # =========================================================
            def attn_phase(l, xbufs, ar_site):
                # KT per head / V built first (k, v, then per-head q + attn)
                KT = sb.tile([128, HL, T], dt.float16, tag="KT", bufs=1,
                             name=f"KT{l}")
                Vt = sb.tile([128, HL, NTT, DH], dt.float16, tag="V", bufs=1,
                             name=f"V{l}")
                nc.sync.dma_start(KT[:, :, :M], mkTd[l][:])
                nc.sync.dma_start(Vt[:, :, :NMT, :], mvd[l][:])
                # k/v projections, chunk-major so chunk 0 streams while
                # chunk 1's AR + rms still run
                for ch in range(NCH):
                    for half in range(2):
                        wk = sb.tile([128, C, 256], dt.float16, tag="wp", bufs=2,
                                     name=f"wk{l}{ch}{half}")
                        nc.sync.dma_start(wk[:], wqkvT[l, 1, half])
                        for dd in range(2):
                            d = 2 * half + dd
                            acc = mm_ps(f"k{l}{d}{ch}")
                            for c in range(C):
                                nc.tensor.matmul(
                                    acc[:], wk[:, c, 128 * dd:128 * (dd + 1)],
                                    xbufs[ch][:, c, :], start=(c == 0),
                                    stop=(c == C - 1))
                            rope_apply(acc, kc[:, 0, M + SC * ch:M + SC * (ch + 1)],
                                       kc[:, 1, M + SC * ch:M + SC * (ch + 1)],
                                       KT[:, d, M + SC * ch:M + SC * (ch + 1)])
                    for half in range(2):
                        wv = sb.tile([128, C, 256], dt.float16, tag="wp", bufs=2,
                                     name=f"wv{l}{ch}{half}")
                        nc.sync.dma_start(wv[:], wqkvT[l, 2, half])
                        for sti in range(4):
                            st = 4 * ch + sti
                            acc = mm_ps(f"v{l}{half}{st}")
                            for c in range(C):
                                nc.tensor.matmul(
                                    acc[:, :256],
                                    xbufs[ch][:, c, 128 * sti:128 * (sti + 1)],
                                    wv[:, c, :], start=(c == 0), stop=(c == C - 1))
                            nc.vector.tensor_copy(
                                Vt[:, 2 * half:2 * half + 2, NMT + st, :],
                                acc[:, :256].rearrange("p (h d) -> p h d", d=128))
                # per-head: q proj + attention
                oT = sb.tile([128, HL, S], dt.float16, tag="oT", bufs=1,
                             name=f"oT{l}")
                for half in range(2):
                    wqh = sb.tile([128, C, 256], dt.float16, tag="wp", bufs=2,
                                  name=f"wq{l}{half}")
                    nc.sync.dma_start(wqh[:], wqkvT[l, 0, half])
                    for hh in range(2):
                        h = 2 * half + hh
                        qT = sb.tile([128, S], dt.float16, tag="qT", bufs=2,
                                     name=f"qT{l}{h}")
                        for ch in range(NCH):
                            acc = mm_ps(f"q{l}{h}{ch}")
                            for c in range(C):
                                nc.tensor.matmul(
                                    acc[:], wqh[:, c, 128 * hh:128 * (hh + 1)],
                                    xbufs[ch][:, c, :], start=(c == 0),
                                    stop=(c == C - 1))
                            rope_apply(acc, qc[:, 0, SC * ch:SC * (ch + 1)],
                                       qc[:, 1, SC * ch:SC * (ch + 1)],
                                       qT[:, SC * ch:SC * (ch + 1)])
                        for sb_i in range(NCH):
                            ntt = NMT + 4 * (sb_i + 1)
                            o_ps = mm_ps(f"o{l}{h}{sb_i}")
                            s_ps = aux_ps(f"s{l}{h}{sb_i}")
                            for tt in range(ntt):
                                sc_ps = mm_ps(f"sc{l}{h}{sb_i}{tt}")
                                nc.tensor.matmul(sc_ps[:],
                                                 KT[:, h, 128 * tt:128 * (tt + 1)],
                                                 qT[:, SC * sb_i:SC * (sb_i + 1)],
                                                 start=True, stop=True)
                                dtile = tt - ntt + 4      # >= 0 -> diagonal tile
                                if dtile >= 0:
                                    off = 384 - 128 * dtile
                                    nc.vector.tensor_tensor(
                                        sc_ps[:], sc_ps[:],
                                        mask[:, off:off + 512], ALU.add)
                                pt = sb.tile([128, 512], dt.float16, tag="pt",
                                             bufs=3, name="pt")
                                nc.scalar.activation(pt[:], sc_ps[:], AF.Exp,
                                                     scale=SCALE)
                                nc.tensor.matmul(o_ps[:], Vt[:, h, tt, :], pt[:],
                                                 start=(tt == 0),
                                                 stop=(tt == ntt - 1))
                                nc.tensor.matmul(s_ps[:], ones_bf[:], pt[:],
                                                 start=(tt == 0),
                                                 stop=(tt == ntt - 1))
                            rrf = sb.tile([1, 512], dt.float32, tag="rowf", bufs=2,
                                          name="rrf")
                            nc.vector.reciprocal(rrf[:], s_ps[:])
                            rr = sb.tile([1, 512], dt.float16, tag="row2", bufs=2,
                                         name="rr")
                            with nc.allow_low_precision(reason="fp16 row for broadcast mm"):
                                nc.vector.tensor_scalar_mul(rr[:], rrf[:], 64.0)
                            bc = ps.tile([128, 512], dt.float32, tag="bc", bufs=2,
                                         name="bca")
                            nc.tensor.matmul(bc[:], inv64_row[:], rr[:],
                                             start=True, stop=True)
                            bcs = sb.tile([128, 512], dt.float32, tag="bcs",
                                          bufs=1, name="bcs")
                            nc.vector.tensor_copy(bcs[:], bc[:])
                            nc.vector.tensor_tensor(
                                oT[:, h, SC * sb_i:SC * (sb_i + 1)],
                                o_ps[:], bcs[:], ALU.mult)
                # Wo: out [Do, s] partial sums -> arin
                for ch in range(NCH):
                  for half in range(2):
                    wo = sb.tile([128, HL, 2048], dt.float16, tag="wp", bufs=2,
                                 name=f"wo{l}{ch}{half}")
                    nc.sync.dma_start(wo[:], woT[l, half])
                    for do in range(16):
                        if True:
                            acc = mm_ps(f"wo{l}{half}{do}{ch}")
                            for hh in range(HL):
                                nc.tensor.matmul(
                                    acc[:], wo[:, hh, 128 * do:128 * (do + 1)],
                                    oT[:, hh, SC * ch:SC * (ch + 1)],
                                    start=(hh == 0), stop=(hh == HL - 1))
                            ev = evh("woev")
                            nc.vector.tensor_copy(ev[:], acc[:])
                            nc.sync.dma_start(
                                arin[ar_site][ch]
                                .rearrange("(t p) s -> p t s", p=128)
                                [:, 16 * half + do, :], ev[:])
                  nc.gpsimd.collective_compute(
                      "AllReduce", ALU.add,
                      replica_groups=[list(range(NCORES))],
                      ins=[arin[ar_site][ch][:]], outs=[arout[ar_site][ch][:]])

            # =========================================================
            # FFN for one layer: xbufs -> partial down-proj -> arin
            # =========================================================
            def ffn_phase(l, xbufs, ar_site):
                actT = sb.tile([128, FL // 128, S], dt.float16, tag="actT",
                               bufs=1, name=f"actT{l}")
                for ch in range(NCH):
                    for fe in range(FL // 128):
                        wg = sb.tile([128, C, 128], dt.float16, tag="wp", bufs=2,
                                     name=f"wg{l}{ch}{fe}")
                        nc.sync.dma_start(wg[:], wguT[l, 0, fe])
                        gs = sb.tile([128, 512], dt.float16, tag="gs", bufs=2,
                                     name="gs")
                        acc = mm_ps(f"g{l}{fe}{ch}")
                        for c in range(C):
                            nc.tensor.matmul(acc[:], wg[:, c, :],
                                             xbufs[ch][:, c, :], start=(c == 0),
                                             stop=(c == C - 1))
                        nc.scalar.activation(gs[:], acc[:], AF.Silu)
                        wu = sb.tile([128, C, 128], dt.float16, tag="wp", bufs=2,
                                     name=f"wu{l}{ch}{fe}")
                        nc.sync.dma_start(wu[:], wguT[l, 1, fe])
                        acc2 = mm_ps(f"u{l}{fe}{ch}")
                        for c in range(C):
                            nc.tensor.matmul(acc2[:], wu[:, c, :],
                                             xbufs[ch][:, c, :], start=(c == 0),
                                             stop=(c == C - 1))
                        nc.vector.tensor_tensor(
                            actT[:, fe, SC * ch:SC * (ch + 1)], acc2[:],
                            gs[:], ALU.mult)
                # down proj
                for ch in range(NCH):
                  for quarter in range(4):
                    wd = sb.tile([128, FL // 128, 1024], dt.float16, tag="wp",
                                 bufs=2, name=f"wd{l}{ch}{quarter}")
                    nc.sync.dma_start(wd[:], wdT[l, quarter])
                    for do in range(8):
                        if True:
                            acc = mm_ps(f"wd{l}{quarter}{do}{ch}")
                            for fc in range(FL // 128):
                                nc.tensor.matmul(
                                    acc[:], wd[:, fc, 128 * do:128 * (do + 1)],
                                    actT[:, fc, SC * ch:SC * (ch + 1)],
                                    start=(fc == 0), stop=(fc == FL // 128 - 1))
                            ev = evh("wdev")
                            nc.vector.tensor_copy(ev[:], acc[:])
                            nc.sync.dma_start(
                                arin[ar_site][ch]
                                .rearrange("(t p) s -> p t s", p=128)
                                [:, 8 * quarter + do, :], ev[:])
                  nc.gpsimd.collective_compute(
                      "AllReduce", ALU.add,
                      replica_groups=[list(range(NCORES))],
                      ins=[arin[ar_site][ch][:]], outs=[arout[ar_site][ch][:]])

            # =========================================================
            # main flow
            # =========================================================
            def xb_tiles(nm):
                return [sb.tile([128, C, SC], dt.float16, tag="xb", bufs=2,
                                name=f"{nm}{ch}") for ch in range(NCH)]

            # layer 0
            x0 = xb_tiles("x0")
            rms_pass(h0T[:], None, None, 0, x0, "r0")
            attn_phase(0, x0, 0)
            x1 = xb_tiles("x1")
            rms_pass(h0T[:], arout[0], hdr[0][:], 1, x1, "r1")
            ffn_phase(0, x1, 1)
            # layer 1
            x2 = xb_tiles("x2")
            rms_pass(hdr[0][:], arout[1], hdr[1][:], 2, x2, "r2")
            attn_phase(1, x2, 2)
            x3 = xb_tiles("x3")
            rms_pass(hdr[1][:], arout[2], hdr[2][:], 3, x3, "r3")
            ffn_phase(1, x3, 3)
            # final rms + lm head
            xf = xb_tiles("xf")
            rms_pass(hdr[2][:], arout[3], None, 4, xf, "rf")
            for ch in range(NCH):
              for vq in range(4):
                lm = sb.tile([128, C, 256], dt.float16, tag="wp", bufs=2,
                             name=f"lm{ch}{vq}")
                nc.sync.dma_start(lm[:], lmT[vq])
                for vv in range(2):
                    if True:
                        acc = mm_ps(f"lm{vq}{vv}{ch}")
                        for c in range(C):
                            nc.tensor.matmul(acc[:], lm[:, c, 128 * vv:128 * (vv + 1)],
                                             xf[ch][:, c, :], start=(c == 0),
                                             stop=(c == C - 1))
                        ev = evf("lmev")
                        nc.vector.tensor_copy(ev[:], acc[:])
                        nc.sync.dma_start(
                            logitsT[:].rearrange("(t p) s -> p t s", p=128)
                            [:, 2 * vq + vv, SC * ch:SC * (ch + 1)], ev[:])

    nc.finalize()
    return nc


_NC_CACHE = {}


def _get_module():
    if "nc" not in _NC_CACHE:
        _NC_CACHE["nc"] = build_module()
    return _NC_CACHE["nc"]


def _rope_tables():
    inv_freq = 1.0 / (ROPE_BASE ** (np.arange(0, DH, 2, dtype=np.float64) / DH))
    ang = np.arange(T, dtype=np.float64)[:, None] * inv_freq[None, :]
    emb = np.concatenate([ang, ang], axis=-1)          # [T, DH]
    return np.cos(emb).astype(np.float32), np.sin(emb).astype(np.float32)


def kernel(input_ids, memory, embed, Wq, Wk, Wv, Wo, Wg, Wu, Wd, Wmk, Wmv,
           ln1, ln2, normw, lm_head):
    input_ids = np.asarray(input_ids)
    f32 = np.float32
    memory = np.asarray(memory, f32)

    nc = _get_module()

    # host prep: embedding gather (pure data movement) + layout transforms
    h0 = np.asarray(embed, f32)[input_ids.reshape(-1)]          # [S, D]
    h0T = np.ascontiguousarray(h0.T).astype(BF)                 # [D, S] fp16

    cos, sin = _rope_tables()
    qcs = np.stack([cos[M:], sin[M:]]).transpose(2, 0, 1)       # [128, 2, S]
    kcs = np.stack([cos, sin]).transpose(2, 0, 1)               # [128, 2, T]

    rmat = np.zeros((128, 128), f32)
    for d in range(64):
        rmat[d + 64, d] = -1.0
        rmat[d, d + 64] = 1.0

    tmaskv = np.full((128, 896), NEG, f32)
    for t in range(128):
        tmaskv[t, 384 + t:] = 0.0

    def bf(x):
        return np.ascontiguousarray(x).astype(BF)

    def swz(wT, nsplit):
        """[Din, n] (Din = c*128) -> [nsplit, 128, c, n/nsplit]."""
        c = wT.shape[0] // 128
        n = wT.shape[1]
        w = wT.reshape(c, 128, n).transpose(1, 0, 2)            # [128, c, n]
        w = w.reshape(128, c, nsplit, n // nsplit).transpose(2, 0, 1, 3)
        return w

    memT = np.stack([swz(memory[l, 0].T, 1)[0] for l in range(L)])

    in_maps = []
    for i in range(NCORES):
        hs = slice(DL * i, DL * (i + 1))
        fs = slice(FL * i, FL * (i + 1))
        vs = slice(VL * i, VL * (i + 1))
        lnw = np.stack([np.asarray(ln1, f32)[0], np.asarray(ln2, f32)[0],
                        np.asarray(ln1, f32)[1], np.asarray(ln2, f32)[1],
                        np.asarray(normw, f32)])                # [5, D]
        in_maps.append({
            "h0T": h0T,
            "memT": bf(memT),
            "wqkvT": bf(np.stack([np.stack([swz(np.asarray(W, f32)[l][hs].T, 2)
                                            for W in (Wq, Wk, Wv)])
                                  for l in range(L)])),
            "wmT": bf(np.stack([np.stack([swz(np.asarray(W, f32)[l][hs].T, 2)
                                          for W in (Wmk, Wmv)])
                                for l in range(L)])),
            "woT": bf(np.stack([swz(np.asarray(Wo, f32)[l][:, hs].T, 2)
                                for l in range(L)])),
            "wguT": bf(np.stack([np.stack([swz(np.asarray(W, f32)[l][fs].T, 8)
                                           for W in (Wg, Wu)])
                                 for l in range(L)])),
            "wdT": bf(np.stack([swz(np.asarray(Wd, f32)[l][:, fs].T, 4)
                                for l in range(L)])),
            "lmT": bf(swz(np.asarray(lm_head, f32)[vs].T, 4)),
            "qcs": bf(qcs),
            "kcs": bf(kcs),
            "rmat": bf(rmat),
            "tmask": bf(tmaskv),
            "lnw": np.ascontiguousarray(
                lnw.reshape(5, C, 128).transpose(2, 0, 1)),     # [128, 5, C]
        })

    res = run_bass_kernel_spmd(nc, in_maps, core_ids=list(range(NCORES)))
    _NC_CACHE["last_results"] = res

    logits = np.empty((B, S, V), f32)
    for i in range(NCORES):
        logits[0, :, VL * i:VL * (i + 1)] = res.results[i]["logitsT"].T
    return logits
